# revision 40
# baseline (speedup 1.0000x reference)
"""Distributed Trainium2 kernel for 16-head causal attention with RoPE.

B=2, S=2048, D=2048, H=16, HD=128. Tensor-parallel over heads: core c owns
heads {2c, 2c+1}. Each core computes q/k/v projections for its heads,
RoPE, causal attention, and a partial output projection (wo row-shard);
the host sums the 8 partials (the unshard step for a row-sharded wo).

Device-side layout choices (all transposes are done on the host):
  - x is fed pre-transposed as xt[d, tok] so every matmul contracts over
    the partition axis with no on-device transposes.
  - q/k are produced head-dim-major (qT[hd, tok]); the RoPE even/odd pair
    permutation is folded into the wq/wk columns on the host, so RoPE is
    six plain elementwise ops on [64, tok] slices.
  - scores are computed transposed (scoresT[k, q]) in 128-wide k chunks,
    software-pipelined 3 deep so the PE never waits on the exp chain.
    The diagonal 128-blocks are triangularly trimmed (q range restricted
    per block). Softmax sums over k accumulate on DVE (bf16 adds), one
    ones-matmul per (b,h,qt) reduces+broadcasts, and a DVE divide
    normalizes (no Ln/Exp round-trip on the ACT engine).
  - attention output oT[hd, q] is exactly the lhsT the output projection
    needs, so the whole pipeline has zero on-device transposes.
  - out partials are cast to fp16, staged [128, 2048] per token block,
    and stored with one DMA per block (host sums partials in fp32).
"""

import numpy as np
from contextlib import ExitStack

B, S, D = 2, 2048, 2048
H, HD, HALF = 16, 128, 64
BS = B * S
NCORES = 8
HPC = H // NCORES          # heads per core
TT = 512                   # token tile for projections
QT = 512                   # q tile in attention
KC = 128                   # k chunk in attention
NKT = D // 128             # 16 contraction chunks of the model dim
NTT = BS // TT             # 8 token tiles
ISQRT = 1.0 / float(np.sqrt(HD))


def _legalize_waits(bir: bytes) -> bytes:
    """Split multi-wait sync_info into standalone EventSemaphore instructions.

    The neuronxcc walrus codegen only encodes ONE sync wait slot on compute
    instructions (Matmult/TensorTensor/...); Tile's sem-assignment freely
    emits several. Hoisting the extras into same-engine EventSemaphore
    instructions placed immediately before the consumer is semantically
    identical (the sequencer blocks on them in program order).
    """
    import json

    d = json.loads(bir)
    wid = 0
    for fn in d["functions"]:
        for blk in fn["blocks"]:
            out = []
            for inst in blk["instructions"]:
                si = inst.get("sync_info")
                if si:
                    waits = si.get("on_wait") or []
                    if len(waits) > 1 and inst.get("engine") not in (None, "Unassigned"):
                        for w in waits[:-1]:
                            wid += 1
                            out.append(
                                {
                                    "debug": inst.get("debug", 0),
                                    "engine": inst["engine"],
                                    "ins": [],
                                    "name": f"hoisted-wait-{wid}",
                                    "opcode": "EventSemaphore",
                                    "outs": [],
                                    "sync_info": {"on_update": [], "on_wait": [w]},
                                }
                            )
                        si["on_wait"] = [waits[-1]]
                out.append(inst)
            blk["instructions"] = out
    return json.dumps(d).encode()


def _patch_serialization(nc):
    import types

    orig = nc.to_json_bytes

    def patched(self):
        return _legalize_waits(orig())

    nc.to_json_bytes = types.MethodType(patched, nc)
    return nc


def _build_nc():
    import concourse.bass as bass
    import concourse.tile as tile
    from concourse import mybir

    f32 = mybir.dt.float32
    f16 = mybir.dt.float16
    bf16 = mybir.dt.bfloat16
    Exp = mybir.ActivationFunctionType.Exp
    Ln = mybir.ActivationFunctionType.Ln
    mult = mybir.AluOpType.mult
    sub = mybir.AluOpType.subtract
    add = mybir.AluOpType.add

    nc = bass.Bass()

    # All weight/activation DRAM params are FLAT 2D (or tile-major 3D for
    # xt) so multi-chunk DMA pieces lower to 4-16KB contiguous elements.
    # The SDMA rings drain ~100GB/s at 1KB elements but ~300GB/s at 4KB+,
    # and the whole early phase is delivery-bound.
    WCOLS = NKT * 2 * HD
    XCOLS = NKT * TT
    xt_h = nc.declare_dram_parameter("xt", [128, NTT, XCOLS], bf16, isOutput=False)
    wq_h = nc.declare_dram_parameter("wq", [128, WCOLS], bf16, isOutput=False)
    wk_h = nc.declare_dram_parameter("wk", [128, WCOLS], bf16, isOutput=False)
    wv_h = nc.declare_dram_parameter("wv", [128, WCOLS], bf16, isOutput=False)
    wo_h = nc.declare_dram_parameter("wo", [128, 2 * D], bf16, isOutput=False)
    cs_h = nc.declare_dram_parameter("cs", [128, BS], bf16, isOutput=False)
    m4_h = nc.declare_dram_parameter("m4", [128, 1280], bf16, isOutput=False)
    out_h = nc.declare_dram_parameter("out", [BS, D], f16, isOutput=True)

    with ExitStack() as ctx:
        tc = ctx.enter_context(tile.TileContext(nc))
        const = ctx.enter_context(tc.tile_pool(name="const", bufs=1))
        persist = ctx.enter_context(tc.tile_pool(name="persist", bufs=1))
        xtp = ctx.enter_context(tc.tile_pool(name="xtp", bufs=4))
        expp = ctx.enter_context(tc.tile_pool(name="expp", bufs=4))
        esp = ctx.enter_context(tc.tile_pool(name="esp", bufs=3))
        ropet = ctx.enter_context(tc.tile_pool(name="ropet", bufs=6))
        # outp 3-deep: copies must not back up on out-store DMA completion
        # (a blocked copy sits in the ACT stream ahead of attention exps)
        outp = ctx.enter_context(tc.tile_pool(name="outp", bufs=3))
        psA = ctx.enter_context(tc.tile_pool(name="psA", bufs=2, space="PSUM"))
        psS = ctx.enter_context(tc.tile_pool(name="psS", bufs=2, space="PSUM"))
        # ov double-buffered: the next attention's PV accumulation must not
        # wait for the previous one's normalize chain to release the bank.
        # sm shares psA's slots (it lives ~1.4us, between ones-mm and Ln).
        psO = ctx.enter_context(tc.tile_pool(name="psO", bufs=2, space="PSUM"))

        # ---- constants into SBUF ----
        # Flat 2D tiles mirror the flat DRAM params; matmul operands slice
        # computed column ranges.
        wq_sb = const.tile([128, WCOLS], bf16, tag="wq")
        wk_sb = const.tile([128, WCOLS], bf16, tag="wk")
        wv_sb = const.tile([128, WCOLS], bf16, tag="wv")
        wo_sb = const.tile([128, 2 * D], bf16, tag="wo")
        cs_sb = const.tile([128, BS], bf16, tag="cs")
        m4_sb = const.tile([128, 1280], bf16, tag="m4")
        ones_sb = const.tile([128, 128], bf16, tag="ones")
        # Startup loads are queued deep on both HWDGE rings immediately in
        # need order (the SDMA engines pipeline across queued DMAs, so a
        # deep queue drains faster than issue-as-needed):
        #   sync:   wq + xt0 + xt1 head  -> later all out stores
        #   scalar: cs0, wk, xt1 tail, wv, m4, cs1, xt2, xt3, wo + xt4-7
        # Early compute order is q(t0), k(t0), q(t1), k(t1), v(t0), v(t1).
        # (Tried and rejected: any early SWDGE/gpsimd traffic measurably
        # slows both HWDGE rings; striping pieces across the two rings in
        # strict need order was no better than this split.)
        C4 = 4 * 2 * HD  # 4 contraction chunks of a weight matrix, flat
        X4 = 4 * TT      # 4 contraction chunks of an xt tile, flat
        xt_tiles = [
            xtp.tile([128, XCOLS], bf16, tag="xt", name=f"xt{i}") for i in range(4)
        ]
        # first pieces extra-fine (2 chunks) so the very first q(t0)
        # matmuls start ~2us earlier on the slow-ramping ring
        C2 = 2 * 2 * HD
        X2 = 2 * TT
        nc.sync.dma_start(wq_sb[:, 0:C2], wq_h[:, 0:C2])
        nc.sync.dma_start(xt_tiles[0][:, 0:X2], xt_h[:, 0, 0:X2])
        nc.sync.dma_start(wq_sb[:, C2:C4], wq_h[:, C2:C4])
        nc.sync.dma_start(xt_tiles[0][:, X2:X4], xt_h[:, 0, X2:X4])
        nc.sync.dma_start(wq_sb[:, C4:WCOLS], wq_h[:, C4:WCOLS])
        for p in range(1, 4):
            nc.sync.dma_start(
                xt_tiles[0][:, p * X4 : (p + 1) * X4],
                xt_h[:, 0, p * X4 : (p + 1) * X4],
            )
        for p in range(2):
            nc.sync.dma_start(
                xt_tiles[1][:, p * X4 : (p + 1) * X4],
                xt_h[:, 1, p * X4 : (p + 1) * X4],
            )
        nc.scalar.dma_start(cs_sb[:, 0 : 2 * TT], cs_h[:, 0 : 2 * TT])
        nc.scalar.dma_start(wk_sb[:, 0:C4], wk_h[:, 0:C4])
        nc.scalar.dma_start(wk_sb[:, C4:WCOLS], wk_h[:, C4:WCOLS])
        for p in range(2, 4):
            nc.scalar.dma_start(
                xt_tiles[1][:, p * X4 : (p + 1) * X4],
                xt_h[:, 1, p * X4 : (p + 1) * X4],
            )
        nc.scalar.dma_start(wv_sb[:], wv_h[:])
        nc.scalar.dma_start(m4_sb[:], m4_h[:])
        nc.scalar.dma_start(cs_sb[:, 2 * TT :], cs_h[:, 2 * TT :])
        for t in (2, 3):
            for p in range(2):
                nc.scalar.dma_start(
                    xt_tiles[t][:, p * 2 * X4 : (p + 1) * 2 * X4],
                    xt_h[:, t, p * 2 * X4 : (p + 1) * 2 * X4],
                )
        nc.scalar.dma_start(wo_sb[:], wo_h[:])

        # tiles 4-7 prefetch on scalar, each emitted right after the tile
        # unit that frees its pool slot (so the slot wait is already
        # satisfied and never blocks the ACT stream), keeping the sync ring
        # free for out stores. (SWDGE/gpsimd rings measurably slow the
        # HWDGE rings when used concurrently — keep everything on HWDGE.)
        def prefetch_xt(t):
            xt_t = xtp.tile([128, XCOLS], bf16, tag="xt", name=f"xt{t}")
            assert len(xt_tiles) == t
            xt_tiles.append(xt_t)
            for p in range(2):
                nc.scalar.dma_start(
                    xt_t[:, p * 2 * X4 : (p + 1) * 2 * X4],
                    xt_h[:, t, p * 2 * X4 : (p + 1) * 2 * X4],
                )

        nc.vector.memset(ones_sb[:], 1.0)

        # PE warm-up: the HAM clock gate needs ~3us of sustained matmul
        # activity to lift the PE from 1.2 to 2.4 GHz. The first real
        # matmuls can't start until their DMAs land (~11us), so burn the
        # wait warming the array on the memset ones tile (no DMA deps).
        warm_ps = psS.tile([128, QT], mybir.dt.float32, tag="sc")
        for i in range(16):
            nc.tensor.matmul(
                warm_ps[:, (i % 4) * 128 : (i % 4) * 128 + 128],
                ones_sb[:], ones_sb[:], start=True, stop=True,
            )
        # consume so DCE keeps the warm-up and the PSUM slot is released
        warm_sink = const.tile([1, 8], mybir.dt.float32, tag="wsink")
        nc.scalar.copy(warm_sink[0:1, 0:2], warm_ps[0:1, 0:2])

        # DVE pre-touch of DMA-written constants: TensorTensor instructions
        # encode only one sync-wait slot, so the DVE vector clock must have
        # observed these DMAs before any TT reads them (else walrus dies with
        # "Too many sync wait commands").
        scratch = const.tile([1, 8], bf16, tag="scratch")
        nc.vector.tensor_copy(scratch[0:1, 0:2], cs_sb[0:1, 0:2])
        nc.vector.tensor_copy(scratch[0:1, 2:4], m4_sb[0:1, 0:2])

        # persistent activations
        qr = persist.tile([128, HPC, BS], bf16, tag="qr")   # rotated qT per head
        kr = persist.tile([128, HPC, BS], bf16, tag="kr")   # rotated kT per head
        v_sb = persist.tile([128, BS // 128, 2 * HD], bf16, tag="v")  # tok-major v
        on_sb = persist.tile([128, HPC, B, S], bf16, tag="on")  # normalized oT

        # ---- phase 1 unit: one token tile's q/k/v projections + RoPE ----
        alt = [0]

        def tile_unit(t, parts="qkv"):
            t0 = t * TT
            xt_t = xt_tiles[t]

            # During tiles 0-3 the score pool psS is idle: alternate the
            # projection PSUM between psA and psS so a slow RoPE drain
            # (DVE, gated on the cs DMA) can lag 4 groups behind the PE
            # without starving it of accumulator slots.
            def proj_tile():
                alt[0] += 1
                if alt[0] % 2:
                    return psS.tile([128, TT], mybir.dt.float32, tag="sc", name="pp")
                return psA.tile([128, TT], mybir.dt.float32, tag="proj", name="pp")

            co = cs_sb[0:HALF, t0 : t0 + TT]
            si = cs_sb[HALF:128, t0 : t0 + TT]
            # q for both heads first (only needs wq), then k for both
            # heads: gives the wk DMA (behind wq on sync) time to land
            # without stalling the PE on tile 0.
            wlist = []
            if "q" in parts:
                wlist.append((wq_sb, qr))
            if "k" in parts:
                wlist.append((wk_sb, kr))
            for w_sb, dstT in wlist:
                for h in range(HPC):
                    pq = proj_tile()
                    for c in range(NKT):
                        wc = c * 2 * HD + h * HD
                        nc.tensor.matmul(
                            pq[:],
                            w_sb[:, wc : wc + HD],
                            xt_t[:, c * TT : (c + 1) * TT],
                            start=(c == 0),
                            stop=(c == NKT - 1),
                        )
                    t1 = ropet.tile([HALF, TT], bf16, tag="rt")
                    t2 = ropet.tile([HALF, TT], bf16, tag="rt")
                    t3 = ropet.tile([HALF, TT], bf16, tag="rt")
                    t4 = ropet.tile([HALF, TT], bf16, tag="rt")
                    # all four PSUM-reading muls first so pq's slot frees
                    # as soon as possible (the combines only read SBUF)
                    nc.vector.tensor_tensor(t1[:], pq[0:HALF, :], co, mult)
                    nc.vector.tensor_tensor(t2[:], pq[HALF:128, :], si, mult)
                    nc.vector.tensor_tensor(t3[:], pq[0:HALF, :], si, mult)
                    nc.vector.tensor_tensor(t4[:], pq[HALF:128, :], co, mult)
                    # combines are SBUF-only and far off the critical path
                    # (consumed by attention much later): run them on the
                    # otherwise-idle Pool engine to unload DVE
                    nc.gpsimd.tensor_tensor(
                        dstT[0:HALF, h, t0 : t0 + TT], t1[:], t2[:], sub
                    )
                    nc.gpsimd.tensor_tensor(
                        dstT[HALF:128, h, t0 : t0 + TT], t3[:], t4[:], add
                    )

            # v projection, token-major [tok, 2*HD]
            for m in range(TT // 128 if "v" in parts else 0):
                pv = proj_tile()
                for c in range(NKT):
                    nc.tensor.matmul(
                        pv[:, 0 : 2 * HD],
                        xt_t[:, c * TT + m * 128 : c * TT + (m + 1) * 128],
                        wv_sb[:, c * 2 * HD : (c + 1) * 2 * HD],
                        start=(c == 0),
                        stop=(c == NKT - 1),
                    )
                g = t * (TT // 128) + m
                nc.scalar.copy(v_sb[:, g, :], pv[:, 0 : 2 * HD])

        # out-proj work queue: single [128,512] units threaded into the
        # attention chunk loop so the PE stream never has copy-gated bursts
        pend = []

        def emit_pend(n):
            for _ in range(min(n, len(pend))):
                outproj_one(*pend.pop(0))

        def attn(b, h, qt, reserve=0):
            q0 = b * S + qt * QT
            # k chunks packed two per PSUM tile / exp call (halves the ACT
            # instruction count and its per-call PSUM access penalty).
            # Full-width unmasked pairs below the diagonal first, then the
            # two triangularly-trimmed masked diagonal pairs, whose chunks
            # pack at column offsets c0 matching the m4 mask layout.
            pairs = [
                ([(2 * p, 0, QT, 0), (2 * p + 1, 0, QT, QT)], None)
                for p in range(2 * qt)
            ]
            pairs.append(
                ([(4 * qt, 0, QT, 0), (4 * qt + 1, 128, QT - 128, QT)], 0)
            )
            pairs.append(
                ([(4 * qt + 2, 256, QT - 256, 0),
                  (4 * qt + 3, 384, QT - 384, QT - 256)], 896)
            )
            n = 4 * qt + 4
            ov = psO.tile([128, QT], mybir.dt.float32, tag="ov")
            es = esp.tile([128, QT], bf16, tag="es")
            meta = []
            flat = [0]

            def emit_pair(pcs, moff):
                wtot = pcs[-1][3] + pcs[-1][2]
                sc = psS.tile([128, 2 * QT], mybir.dt.float32, tag="sc", name="sc")
                for kb, qoff, w, c0 in pcs:
                    k0 = b * S + kb * KC
                    nc.tensor.matmul(
                        sc[:, c0 : c0 + w],
                        kr[:, h, k0 : k0 + KC],
                        qr[:, h, q0 + qoff : q0 + QT],
                        start=True,
                        stop=True,
                    )
                e = expp.tile([128, 2 * QT], bf16, tag="e", name="e")
                nc.scalar.activation(e[:, 0:wtot], sc[:, 0:wtot], Exp, scale=ISQRT)
                if moff is not None:
                    nc.vector.tensor_tensor(
                        e[:, 0:wtot], e[:, 0:wtot], m4_sb[:, moff : moff + wtot], mult
                    )
                ms = []
                for kb, qoff, w, c0 in pcs:
                    if flat[0] == 0:
                        nc.vector.tensor_copy(es[:], e[:, 0:QT])
                    else:
                        nc.vector.tensor_tensor(
                            es[:, qoff:QT], es[:, qoff:QT], e[:, c0 : c0 + w], add
                        )
                    ms.append((kb, qoff, w, c0, flat[0]))
                    flat[0] += 1
                meta.append((e, ms))

            def do_pv(pi):
                e, ms = meta[pi]
                for kb, qoff, w, c0, ci in ms:
                    gk = (b * S + kb * KC) // 128
                    nc.tensor.matmul(
                        ov[:, qoff:QT],
                        v_sb[:, gk, h * HD : (h + 1) * HD],
                        e[:, c0 : c0 + w],
                        start=(ci == 0),
                        stop=(ci == n - 1),
                        skip_group_check=True,
                    )

            # PV lags TWO pairs behind the score/exp front (expp holds 4
            # e tiles), so a PV matmul's exp+mask have ~2 pair-times of
            # slack and the PE stops catching the ACT chain's tail
            for pi, (pcs, moff) in enumerate(pairs):
                emit_pair(pcs, moff)
                if pi >= 2:
                    do_pv(pi - 2)
                if len(pend) > reserve:
                    emit_pend(min(2, len(pend) - reserve))
            if len(pairs) >= 2:
                do_pv(len(pairs) - 2)
            do_pv(len(pairs) - 1)
            # sum over k (partition axis) via ones-matmul: reduces and
            # broadcasts the denominator to all 128 partitions in one shot
            sm = psA.tile([128, QT], mybir.dt.float32, tag="proj", name="sm")
            nc.tensor.matmul(sm[:], ones_sb[:], es[:], start=True, stop=True)
            # fill the serial ones-mm -> Ln -> Exp -> mult latency with any
            # queued out-proj units (they depend only on earlier attns)
            emit_pend(4)
            # 1/sum via exp(-ln(sum)): Ln and Exp share one ACT table so no
            # table reloads (DVE has no divide, and TT can't read 2x PSUM).
            lnt = esp.tile([128, QT], mybir.dt.float32, tag="smsb")
            nc.scalar.activation(lnt[:], sm[:], Ln)
            rr = esp.tile([128, QT], mybir.dt.float32, tag="smsb")
            nc.scalar.activation(rr[:], lnt[:], Exp, scale=-1.0)
            # pre-touch rr on DVE so the norm TT only waits on PE
            nc.vector.tensor_copy(scratch[0:1, 4:6], rr[0:1, 0:2])
            nc.vector.tensor_tensor(
                on_sb[:, h, b, qt * QT : qt * QT + QT], ov[:], rr[:], mult
            )

        ecount = 0
        cur_stage = [None]

        def outproj_one(b, tcn, et):
            nonlocal ecount
            po = psA.tile([128, 512], mybir.dt.float32, tag="proj")
            for j in range(HPC):
                nc.tensor.matmul(
                    po[:],
                    on_sb[:, j, b, tcn * 128 : tcn * 128 + 128],
                    wo_sb[:, j * D + et * 512 : j * D + et * 512 + 512],
                    start=(j == 0),
                    stop=(j == HPC - 1),
                )
            if et == 0:
                cur_stage[0] = outp.tile([128, D], f16, tag="ob", name="ob")
            ob = cur_stage[0]
            eng = (nc.scalar.copy, nc.vector.tensor_copy)[ecount % 2]
            eng(ob[:, et * 512 : et * 512 + 512], po[:])
            ecount += 1
            rows = slice(b * S + tcn * 128, b * S + tcn * 128 + 128)
            if b == 1 and tcn >= 12:
                # tail-critical blocks: store per half; the final block
                # per-unit so the very last transfer is only 128KB
                if tcn == 15:
                    cols = slice(et * 512, et * 512 + 512)
                    nc.sync.dma_start(out_h[rows, cols], ob[:, cols])
                elif et == 1 or et == 3:
                    cols = slice(0, 1024) if et == 1 else slice(1024, 2048)
                    nc.sync.dma_start(out_h[rows, cols], ob[:, cols])
            elif et == 3:
                # one full-width store per token block: 4KB contiguous rows
                # drain the ring at roughly twice the rate of 2KB halves
                nc.sync.dma_start(out_h[rows, :], ob[:, :])

        # Batch-0 token tiles first; then batch-1 tiles with batch-0's
        # attention interleaved between them (its exp load lands where
        # ScalarE is otherwise idle, and it needs no PSUM "proj" slots so
        # it doesn't fight the projection groups). Out-proj units run as
        # single [128,512] groups threaded into the batch-1 attention
        # chunk loops (emit_pend inside attn), never in bursts.
        # Batch-0 attention q-tile t only needs tokens [0, (t+1)*512) of
        # q/k/v — run it right after token tile t, where its matmuls and
        # exps fill the startup DMA-delivery stalls. Tiles 4-7 (batch 1
        # projections) then run as one PE-saturated stretch, draining
        # queued out-proj units into any gaps.
        # tiles 0-1 run q(t0), k(t0), q(t1), k(t1): tile 0's work only
        # needs wq/wk/xt0 (first on both rings), so the PE never idles on
        # xt1's tail the way q(t0),q(t1) back-to-back would
        tile_unit(0, parts="q")
        tile_unit(0, parts="k")
        tile_unit(1, parts="q")
        tile_unit(1, parts="k")
        tile_unit(0, parts="v")
        prefetch_xt(4)
        tile_unit(1, parts="v")
        prefetch_xt(5)
        for t in range(2):
            attn(0, 0, t, reserve=6)
            attn(0, 1, t, reserve=6)
            pend += [(0, 4 * t + i, et) for i in range(4) for et in range(D // 512)]
        for t in range(2, NTT // 2):
            tile_unit(t)
            prefetch_xt(t + 4)
            attn(0, 0, t, reserve=6)
            attn(0, 1, t, reserve=6)
            pend += [(0, 4 * t + i, et) for i in range(4) for et in range(D // 512)]
        for t in range(NTT // 2, NTT):
            tile_unit(t)
            emit_pend(4)
        # a standing reserve of units keeps the PE fed through every
        # ones-mm -> Ln -> Exp -> mult boundary chain in the b1 phase
        for qt in range(S // QT):
            rsv = 14 if qt == S // QT - 1 else 12
            attn(1, 0, qt, reserve=rsv)
            emit_pend(2)
            attn(1, 1, qt, reserve=rsv)
            emit_pend(2)
            pend += [(1, 4 * qt + i, et) for i in range(4) for et in range(D // 512)]
        emit_pend(len(pend))
    return _patch_serialization(nc)


def _prep_inputs(x, wq, wk, wv, wo, freqs_cos, freqs_sin):
    import ml_dtypes

    bf16 = ml_dtypes.bfloat16
    perm = np.concatenate([np.arange(0, HD, 2), np.arange(1, HD, 2)])

    xt = np.ascontiguousarray(x.reshape(BS, D).T)          # [D, BS]
    # tile-major pack: [128, NTT, NKT*TT] so a multi-chunk DMA piece of one
    # token tile is contiguous per partition (4-16KB DMA elements)
    xt_r = np.ascontiguousarray(
        xt.reshape(NKT, 128, NTT, TT).transpose(1, 2, 0, 3).reshape(
            128, NTT, NKT * TT
        )
    ).astype(bf16)

    cosT = freqs_cos.T.astype(np.float32)                  # [64, S]
    sinT = freqs_sin.T.astype(np.float32)
    cs = np.concatenate(
        [np.tile(cosT, (1, B)), np.tile(sinT, (1, B))], axis=0
    ).astype(bf16)                                         # [128, BS]

    i = np.arange(KC)[:, None]
    m4 = np.concatenate(
        [(i <= np.arange(w)[None, :]).astype(np.float32) for w in (512, 384, 256, 128)],
        axis=1,
    ).astype(bf16)                                         # [128, 1280] packed triangles

    def pack_w(wmat_cols):
        # wmat_cols: [D, 2*HD] -> [128, NKT * 2*HD] (flat chunk-major)
        return np.ascontiguousarray(
            wmat_cols.reshape(NKT, 128, 2 * HD).transpose(1, 0, 2).reshape(
                128, NKT * 2 * HD
            )
        ).astype(bf16)

    in_maps = []
    for c in range(NCORES):
        heads = [HPC * c + hh for hh in range(HPC)]
        wq_c = np.concatenate(
            [wq[h * HD : (h + 1) * HD][perm].T for h in heads], axis=1
        )                                                  # [D, 2*HD]
        wk_c = np.concatenate(
            [wk[h * HD : (h + 1) * HD][perm].T for h in heads], axis=1
        )
        wv_c = np.concatenate(
            [wv[h * HD : (h + 1) * HD].T for h in heads], axis=1
        )
        wo_c = np.stack(
            [wo[:, h * HD : (h + 1) * HD].T for h in heads], axis=0
        )                                                  # [2, HD, D]
        wo_r = np.ascontiguousarray(wo_c.transpose(1, 0, 2)).astype(bf16).reshape(
            128, 2 * D
        )
        in_maps.append(
            dict(
                xt=xt_r,
                wq=pack_w(wq_c),
                wk=pack_w(wk_c),
                wv=pack_w(wv_c),
                wo=wo_r,
                cs=cs,
                m4=m4,
            )
        )
    return in_maps


_NC_CACHE = {}


def kernel(x, wq, wk, wv, wo, freqs_cos, freqs_sin, mask):
    from concourse.bass_utils import run_bass_kernel_spmd

    in_maps = _prep_inputs(x, wq, wk, wv, wo, freqs_cos, freqs_sin)
    if "nc" not in _NC_CACHE:
        _NC_CACHE["nc"] = _build_nc()
    nc = _NC_CACHE["nc"]
    res = run_bass_kernel_spmd(nc, in_maps, core_ids=list(range(NCORES)))
    parts = [r["out"].astype(np.float32) for r in res.results]
    out = np.sum(np.stack(parts, 0), axis=0, dtype=np.float32)
    return out.reshape(B, S, D)



# revision 41
# speedup vs baseline: 1.0289x; 1.0289x over previous
"""Distributed Trainium2 kernel for 16-head causal attention with RoPE.

B=2, S=2048, D=2048, H=16, HD=128. Tensor-parallel over heads: core c owns
heads {2c, 2c+1}. Each core computes q/k/v projections for its heads,
RoPE, causal attention, and a partial output projection (wo row-shard);
the host sums the 8 partials (the unshard step for a row-sharded wo).

Device-side layout choices (all transposes are done on the host):
  - x is fed pre-transposed as xt[d, tok] so every matmul contracts over
    the partition axis with no on-device transposes.
  - q/k are produced head-dim-major (qT[hd, tok]); the RoPE even/odd pair
    permutation is folded into the wq/wk columns on the host, so RoPE is
    six plain elementwise ops on [64, tok] slices.
  - scores are computed transposed (scoresT[k, q]) in 128-wide k chunks,
    software-pipelined 3 deep so the PE never waits on the exp chain.
    The diagonal 128-blocks are triangularly trimmed (q range restricted
    per block). Softmax sums over k accumulate on DVE (bf16 adds), one
    ones-matmul per (b,h,qt) reduces+broadcasts, and a DVE divide
    normalizes (no Ln/Exp round-trip on the ACT engine).
  - attention output oT[hd, q] is exactly the lhsT the output projection
    needs, so the whole pipeline has zero on-device transposes.
  - out partials are cast to fp16, staged [128, 2048] per token block,
    and stored with one DMA per block (host sums partials in fp32).
"""

import numpy as np
from contextlib import ExitStack

B, S, D = 2, 2048, 2048
H, HD, HALF = 16, 128, 64
BS = B * S
NCORES = 8
HPC = H // NCORES          # heads per core
TT = 512                   # token tile for projections
QT = 512                   # q tile in attention
KC = 128                   # k chunk in attention
NKT = D // 128             # 16 contraction chunks of the model dim
NTT = BS // TT             # 8 token tiles
ISQRT = 1.0 / float(np.sqrt(HD))


def _legalize_waits(bir: bytes) -> bytes:
    """Split multi-wait sync_info into standalone EventSemaphore instructions.

    The neuronxcc walrus codegen only encodes ONE sync wait slot on compute
    instructions (Matmult/TensorTensor/...); Tile's sem-assignment freely
    emits several. Hoisting the extras into same-engine EventSemaphore
    instructions placed immediately before the consumer is semantically
    identical (the sequencer blocks on them in program order).
    """
    import json

    d = json.loads(bir)
    wid = 0
    for fn in d["functions"]:
        for blk in fn["blocks"]:
            out = []
            for inst in blk["instructions"]:
                si = inst.get("sync_info")
                if si:
                    waits = si.get("on_wait") or []
                    if len(waits) > 1 and inst.get("engine") not in (None, "Unassigned"):
                        for w in waits[:-1]:
                            wid += 1
                            out.append(
                                {
                                    "debug": inst.get("debug", 0),
                                    "engine": inst["engine"],
                                    "ins": [],
                                    "name": f"hoisted-wait-{wid}",
                                    "opcode": "EventSemaphore",
                                    "outs": [],
                                    "sync_info": {"on_update": [], "on_wait": [w]},
                                }
                            )
                        si["on_wait"] = [waits[-1]]
                out.append(inst)
            blk["instructions"] = out
    return json.dumps(d).encode()


def _patch_serialization(nc):
    import types

    orig = nc.to_json_bytes

    def patched(self):
        return _legalize_waits(orig())

    nc.to_json_bytes = types.MethodType(patched, nc)
    return nc


def _build_nc():
    import concourse.bass as bass
    import concourse.tile as tile
    from concourse import mybir

    f32 = mybir.dt.float32
    f16 = mybir.dt.float16
    bf16 = mybir.dt.bfloat16
    Exp = mybir.ActivationFunctionType.Exp
    Ln = mybir.ActivationFunctionType.Ln
    mult = mybir.AluOpType.mult
    sub = mybir.AluOpType.subtract
    add = mybir.AluOpType.add

    nc = bass.Bass()

    # All weight/activation DRAM params are FLAT 2D (or tile-major 3D for
    # xt) so multi-chunk DMA pieces lower to 4-16KB contiguous elements.
    # The SDMA rings drain ~100GB/s at 1KB elements but ~300GB/s at 4KB+,
    # and the whole early phase is delivery-bound.
    WCOLS = NKT * 2 * HD
    XCOLS = NKT * TT
    xt_h = nc.declare_dram_parameter("xt", [128, NTT, XCOLS], bf16, isOutput=False)
    wq_h = nc.declare_dram_parameter("wq", [128, WCOLS], bf16, isOutput=False)
    wk_h = nc.declare_dram_parameter("wk", [128, WCOLS], bf16, isOutput=False)
    wv_h = nc.declare_dram_parameter("wv", [128, WCOLS], bf16, isOutput=False)
    wo_h = nc.declare_dram_parameter("wo", [128, 2 * D], bf16, isOutput=False)
    cs_h = nc.declare_dram_parameter("cs", [128, BS], bf16, isOutput=False)
    m4_h = nc.declare_dram_parameter("m4", [128, 1280], bf16, isOutput=False)
    out_h = nc.declare_dram_parameter("out", [BS, D], f16, isOutput=True)

    with ExitStack() as ctx:
        tc = ctx.enter_context(tile.TileContext(nc))
        const = ctx.enter_context(tc.tile_pool(name="const", bufs=1))
        persist = ctx.enter_context(tc.tile_pool(name="persist", bufs=1))
        xtp = ctx.enter_context(tc.tile_pool(name="xtp", bufs=4))
        expp = ctx.enter_context(tc.tile_pool(name="expp", bufs=4))
        esp = ctx.enter_context(tc.tile_pool(name="esp", bufs=3))
        ropet = ctx.enter_context(tc.tile_pool(name="ropet", bufs=6))
        # outp 3-deep: copies must not back up on out-store DMA completion
        # (a blocked copy sits in the ACT stream ahead of attention exps)
        outp = ctx.enter_context(tc.tile_pool(name="outp", bufs=3))
        psA = ctx.enter_context(tc.tile_pool(name="psA", bufs=2, space="PSUM"))
        psS = ctx.enter_context(tc.tile_pool(name="psS", bufs=2, space="PSUM"))
        # ov double-buffered: the next attention's PV accumulation must not
        # wait for the previous one's normalize chain to release the bank.
        # sm shares psA's slots (it lives ~1.4us, between ones-mm and Ln).
        psO = ctx.enter_context(tc.tile_pool(name="psO", bufs=2, space="PSUM"))

        # ---- constants into SBUF ----
        # Flat 2D tiles mirror the flat DRAM params; matmul operands slice
        # computed column ranges.
        wq_sb = const.tile([128, WCOLS], bf16, tag="wq")
        wk_sb = const.tile([128, WCOLS], bf16, tag="wk")
        wv_sb = const.tile([128, WCOLS], bf16, tag="wv")
        wo_sb = const.tile([128, 2 * D], bf16, tag="wo")
        cs_sb = const.tile([128, BS], bf16, tag="cs")
        m4_sb = const.tile([128, 1280], bf16, tag="m4")
        ones_sb = const.tile([128, 128], bf16, tag="ones")
        # Startup loads are queued deep on both HWDGE rings immediately in
        # need order (the SDMA engines pipeline across queued DMAs, so a
        # deep queue drains faster than issue-as-needed). Sync carries the
        # PE-critical wq + xt stream; scalar carries the rest of the
        # weights, then xt tiles 2-3.
        # Early compute order is q(t0), q(t1), k(t0), k(t1), v(t0), v(t1):
        # only wq/xt0/xt1 gate the first ~25us.
        C4 = 4 * 2 * HD  # 4 contraction chunks of a weight matrix, flat
        X4 = 4 * TT      # 4 contraction chunks of an xt tile, flat
        # Three parallel DMA streams (per-ring early throughput is only
        # ~200GB/s, so the 7MB the first 35us of compute needs must be
        # split):
        #   sync (HWDGE):   xt0, xt1 head   -> later all out stores
        #   scalar (HWDGE): wq, cs0, wk, xt1 tail, wv (the weight path)
        #   gpsimd (SWDGE): bulk with slack: m4, cs1, xt2, xt3, wo, xt4-7
        xt_tiles = [
            xtp.tile([128, XCOLS], bf16, tag="xt", name=f"xt{i}") for i in range(4)
        ]
        nc.sync.dma_start(wq_sb[:, 0:C4], wq_h[:, 0:C4])
        nc.sync.dma_start(xt_tiles[0][:, 0:X4], xt_h[:, 0, 0:X4])
        nc.sync.dma_start(wq_sb[:, C4:WCOLS], wq_h[:, C4:WCOLS])
        for p in range(1, 4):
            nc.sync.dma_start(
                xt_tiles[0][:, p * X4 : (p + 1) * X4],
                xt_h[:, 0, p * X4 : (p + 1) * X4],
            )
        for p in range(2):
            nc.sync.dma_start(
                xt_tiles[1][:, p * X4 : (p + 1) * X4],
                xt_h[:, 1, p * X4 : (p + 1) * X4],
            )
        nc.scalar.dma_start(cs_sb[:, 0 : 2 * TT], cs_h[:, 0 : 2 * TT])
        nc.scalar.dma_start(wk_sb[:, 0:C4], wk_h[:, 0:C4])
        nc.scalar.dma_start(wk_sb[:, C4:WCOLS], wk_h[:, C4:WCOLS])
        for p in range(2, 4):
            nc.scalar.dma_start(
                xt_tiles[1][:, p * X4 : (p + 1) * X4],
                xt_h[:, 1, p * X4 : (p + 1) * X4],
            )
        nc.scalar.dma_start(wv_sb[:], wv_h[:])
        nc.scalar.dma_start(m4_sb[:], m4_h[:])
        nc.scalar.dma_start(cs_sb[:, 2 * TT :], cs_h[:, 2 * TT :])
        for t in (2, 3):
            for p in range(2):
                nc.scalar.dma_start(
                    xt_tiles[t][:, p * 2 * X4 : (p + 1) * 2 * X4],
                    xt_h[:, t, p * 2 * X4 : (p + 1) * 2 * X4],
                )
        nc.scalar.dma_start(wo_sb[:], wo_h[:])

        # tiles 4-7 prefetch on scalar, each emitted right after the tile
        # unit that frees its pool slot (so the slot wait is already
        # satisfied and never blocks the ACT stream), keeping the sync ring
        # free for out stores. (SWDGE/gpsimd rings measurably slow the
        # HWDGE rings when used concurrently — keep everything on HWDGE.)
        def prefetch_xt(t):
            xt_t = xtp.tile([128, XCOLS], bf16, tag="xt", name=f"xt{t}")
            assert len(xt_tiles) == t
            xt_tiles.append(xt_t)
            for p in range(2):
                nc.scalar.dma_start(
                    xt_t[:, p * 2 * X4 : (p + 1) * 2 * X4],
                    xt_h[:, t, p * 2 * X4 : (p + 1) * 2 * X4],
                )

        nc.vector.memset(ones_sb[:], 1.0)

        # PE warm-up: the HAM clock gate needs ~3us of sustained matmul
        # activity to lift the PE from 1.2 to 2.4 GHz. The first real
        # matmuls can't start until their DMAs land (~11us), so burn the
        # wait warming the array on the memset ones tile (no DMA deps).
        warm_ps = psS.tile([128, QT], mybir.dt.float32, tag="sc")
        for i in range(16):
            nc.tensor.matmul(
                warm_ps[:, (i % 4) * 128 : (i % 4) * 128 + 128],
                ones_sb[:], ones_sb[:], start=True, stop=True,
            )
        # consume so DCE keeps the warm-up and the PSUM slot is released
        warm_sink = const.tile([1, 8], mybir.dt.float32, tag="wsink")
        nc.scalar.copy(warm_sink[0:1, 0:2], warm_ps[0:1, 0:2])

        # DVE pre-touch of DMA-written constants: TensorTensor instructions
        # encode only one sync-wait slot, so the DVE vector clock must have
        # observed these DMAs before any TT reads them (else walrus dies with
        # "Too many sync wait commands").
        scratch = const.tile([1, 8], bf16, tag="scratch")
        nc.vector.tensor_copy(scratch[0:1, 0:2], cs_sb[0:1, 0:2])
        nc.vector.tensor_copy(scratch[0:1, 2:4], m4_sb[0:1, 0:2])

        # persistent activations
        qr = persist.tile([128, HPC, BS], bf16, tag="qr")   # rotated qT per head
        kr = persist.tile([128, HPC, BS], bf16, tag="kr")   # rotated kT per head
        v_sb = persist.tile([128, BS // 128, 2 * HD], bf16, tag="v")  # tok-major v
        on_sb = persist.tile([128, HPC, B, S], bf16, tag="on")  # normalized oT

        # ---- phase 1 unit: one token tile's q/k/v projections + RoPE ----
        alt = [0]

        def tile_unit(t, parts="qkv"):
            t0 = t * TT
            xt_t = xt_tiles[t]

            # During tiles 0-3 the score pool psS is idle: alternate the
            # projection PSUM between psA and psS so a slow RoPE drain
            # (DVE, gated on the cs DMA) can lag 4 groups behind the PE
            # without starving it of accumulator slots.
            def proj_tile():
                alt[0] += 1
                if alt[0] % 2:
                    return psS.tile([128, TT], mybir.dt.float32, tag="sc", name="pp")
                return psA.tile([128, TT], mybir.dt.float32, tag="proj", name="pp")

            co = cs_sb[0:HALF, t0 : t0 + TT]
            si = cs_sb[HALF:128, t0 : t0 + TT]
            # q for both heads first (only needs wq), then k for both
            # heads: gives the wk DMA (behind wq on sync) time to land
            # without stalling the PE on tile 0.
            wlist = []
            if "q" in parts:
                wlist.append((wq_sb, qr))
            if "k" in parts:
                wlist.append((wk_sb, kr))
            for w_sb, dstT in wlist:
                for h in range(HPC):
                    pq = proj_tile()
                    for c in range(NKT):
                        wc = c * 2 * HD + h * HD
                        nc.tensor.matmul(
                            pq[:],
                            w_sb[:, wc : wc + HD],
                            xt_t[:, c * TT : (c + 1) * TT],
                            start=(c == 0),
                            stop=(c == NKT - 1),
                        )
                    t1 = ropet.tile([HALF, TT], bf16, tag="rt")
                    t2 = ropet.tile([HALF, TT], bf16, tag="rt")
                    t3 = ropet.tile([HALF, TT], bf16, tag="rt")
                    t4 = ropet.tile([HALF, TT], bf16, tag="rt")
                    # all four PSUM-reading muls first so pq's slot frees
                    # as soon as possible (the combines only read SBUF)
                    nc.vector.tensor_tensor(t1[:], pq[0:HALF, :], co, mult)
                    nc.vector.tensor_tensor(t2[:], pq[HALF:128, :], si, mult)
                    nc.vector.tensor_tensor(t3[:], pq[0:HALF, :], si, mult)
                    nc.vector.tensor_tensor(t4[:], pq[HALF:128, :], co, mult)
                    # combines are SBUF-only and far off the critical path
                    # (consumed by attention much later): run them on the
                    # otherwise-idle Pool engine to unload DVE
                    nc.gpsimd.tensor_tensor(
                        dstT[0:HALF, h, t0 : t0 + TT], t1[:], t2[:], sub
                    )
                    nc.gpsimd.tensor_tensor(
                        dstT[HALF:128, h, t0 : t0 + TT], t3[:], t4[:], add
                    )

            # v projection, token-major [tok, 2*HD]
            for m in range(TT // 128 if "v" in parts else 0):
                pv = proj_tile()
                for c in range(NKT):
                    nc.tensor.matmul(
                        pv[:, 0 : 2 * HD],
                        xt_t[:, c * TT + m * 128 : c * TT + (m + 1) * 128],
                        wv_sb[:, c * 2 * HD : (c + 1) * 2 * HD],
                        start=(c == 0),
                        stop=(c == NKT - 1),
                    )
                g = t * (TT // 128) + m
                nc.scalar.copy(v_sb[:, g, :], pv[:, 0 : 2 * HD])

        # out-proj work queue: single [128,512] units threaded into the
        # attention chunk loop so the PE stream never has copy-gated bursts
        pend = []

        def emit_pend(n):
            for _ in range(min(n, len(pend))):
                outproj_one(*pend.pop(0))

        def attn(b, h, qt, reserve=0):
            q0 = b * S + qt * QT
            # k chunks packed two per PSUM tile / exp call (halves the ACT
            # instruction count and its per-call PSUM access penalty).
            # Full-width unmasked pairs below the diagonal first, then the
            # two triangularly-trimmed masked diagonal pairs, whose chunks
            # pack at column offsets c0 matching the m4 mask layout.
            pairs = [
                ([(2 * p, 0, QT, 0), (2 * p + 1, 0, QT, QT)], None)
                for p in range(2 * qt)
            ]
            pairs.append(
                ([(4 * qt, 0, QT, 0), (4 * qt + 1, 128, QT - 128, QT)], 0)
            )
            pairs.append(
                ([(4 * qt + 2, 256, QT - 256, 0),
                  (4 * qt + 3, 384, QT - 384, QT - 256)], 896)
            )
            n = 4 * qt + 4
            ov = psO.tile([128, QT], mybir.dt.float32, tag="ov")
            es = esp.tile([128, QT], bf16, tag="es")
            meta = []
            flat = [0]

            def emit_pair(pcs, moff):
                wtot = pcs[-1][3] + pcs[-1][2]
                sc = psS.tile([128, 2 * QT], mybir.dt.float32, tag="sc", name="sc")
                for kb, qoff, w, c0 in pcs:
                    k0 = b * S + kb * KC
                    nc.tensor.matmul(
                        sc[:, c0 : c0 + w],
                        kr[:, h, k0 : k0 + KC],
                        qr[:, h, q0 + qoff : q0 + QT],
                        start=True,
                        stop=True,
                    )
                e = expp.tile([128, 2 * QT], bf16, tag="e", name="e")
                nc.scalar.activation(e[:, 0:wtot], sc[:, 0:wtot], Exp, scale=ISQRT)
                if moff is not None:
                    nc.vector.tensor_tensor(
                        e[:, 0:wtot], e[:, 0:wtot], m4_sb[:, moff : moff + wtot], mult
                    )
                ms = []
                for kb, qoff, w, c0 in pcs:
                    if flat[0] == 0:
                        nc.vector.tensor_copy(es[:], e[:, 0:QT])
                    else:
                        nc.vector.tensor_tensor(
                            es[:, qoff:QT], es[:, qoff:QT], e[:, c0 : c0 + w], add
                        )
                    ms.append((kb, qoff, w, c0, flat[0]))
                    flat[0] += 1
                meta.append((e, ms))

            def do_pv(pi):
                e, ms = meta[pi]
                for kb, qoff, w, c0, ci in ms:
                    gk = (b * S + kb * KC) // 128
                    nc.tensor.matmul(
                        ov[:, qoff:QT],
                        v_sb[:, gk, h * HD : (h + 1) * HD],
                        e[:, c0 : c0 + w],
                        start=(ci == 0),
                        stop=(ci == n - 1),
                        skip_group_check=True,
                    )

            for pi, (pcs, moff) in enumerate(pairs):
                emit_pair(pcs, moff)
                if pi >= 1:
                    do_pv(pi - 1)
                if len(pend) > reserve:
                    emit_pend(min(2, len(pend) - reserve))
            do_pv(len(pairs) - 1)
            # sum over k (partition axis) via ones-matmul: reduces and
            # broadcasts the denominator to all 128 partitions in one shot
            sm = psA.tile([128, QT], mybir.dt.float32, tag="proj", name="sm")
            nc.tensor.matmul(sm[:], ones_sb[:], es[:], start=True, stop=True)
            # fill the serial ones-mm -> Ln -> Exp -> mult latency with any
            # queued out-proj units (they depend only on earlier attns)
            emit_pend(4)
            # 1/sum via exp(-ln(sum)): Ln and Exp share one ACT table so no
            # table reloads (DVE has no divide, and TT can't read 2x PSUM).
            lnt = esp.tile([128, QT], mybir.dt.float32, tag="smsb")
            nc.scalar.activation(lnt[:], sm[:], Ln)
            rr = esp.tile([128, QT], mybir.dt.float32, tag="smsb")
            nc.scalar.activation(rr[:], lnt[:], Exp, scale=-1.0)
            # pre-touch rr on DVE so the norm TT only waits on PE
            nc.vector.tensor_copy(scratch[0:1, 4:6], rr[0:1, 0:2])
            nc.vector.tensor_tensor(
                on_sb[:, h, b, qt * QT : qt * QT + QT], ov[:], rr[:], mult
            )

        ecount = 0
        cur_stage = [None]

        def outproj_one(b, tcn, et):
            nonlocal ecount
            po = psA.tile([128, 512], mybir.dt.float32, tag="proj")
            for j in range(HPC):
                nc.tensor.matmul(
                    po[:],
                    on_sb[:, j, b, tcn * 128 : tcn * 128 + 128],
                    wo_sb[:, j * D + et * 512 : j * D + et * 512 + 512],
                    start=(j == 0),
                    stop=(j == HPC - 1),
                )
            if et == 0:
                cur_stage[0] = outp.tile([128, D], f16, tag="ob", name="ob")
            ob = cur_stage[0]
            eng = (nc.scalar.copy, nc.vector.tensor_copy)[ecount % 2]
            eng(ob[:, et * 512 : et * 512 + 512], po[:])
            ecount += 1
            rows = slice(b * S + tcn * 128, b * S + tcn * 128 + 128)
            if b == 1 and tcn >= 12:
                # tail-critical blocks: store per half; the final block
                # per-unit so the very last transfer is only 128KB
                if tcn == 15:
                    cols = slice(et * 512, et * 512 + 512)
                    nc.sync.dma_start(out_h[rows, cols], ob[:, cols])
                elif et == 1 or et == 3:
                    cols = slice(0, 1024) if et == 1 else slice(1024, 2048)
                    nc.sync.dma_start(out_h[rows, cols], ob[:, cols])
            elif et == 3:
                # one full-width store per token block: 4KB contiguous rows
                # drain the ring at roughly twice the rate of 2KB halves
                nc.sync.dma_start(out_h[rows, :], ob[:, :])

        # Batch-0 token tiles first; then batch-1 tiles with batch-0's
        # attention interleaved between them (its exp load lands where
        # ScalarE is otherwise idle, and it needs no PSUM "proj" slots so
        # it doesn't fight the projection groups). Out-proj units run as
        # single [128,512] groups threaded into the batch-1 attention
        # chunk loops (emit_pend inside attn), never in bursts.
        # Batch-0 attention q-tile t only needs tokens [0, (t+1)*512) of
        # q/k/v — run it right after token tile t, where its matmuls and
        # exps fill the startup DMA-delivery stalls. Tiles 4-7 (batch 1
        # projections) then run as one PE-saturated stretch, draining
        # queued out-proj units into any gaps.
        # tiles 0-1 run q(t0), k(t0), q(t1), k(t1): tile 0's work only
        # needs wq/wk/xt0 (first on both rings), so the PE never idles on
        # xt1's tail the way q(t0),q(t1) back-to-back would
        tile_unit(0, parts="q")
        tile_unit(0, parts="k")
        tile_unit(1, parts="q")
        tile_unit(1, parts="k")
        tile_unit(0, parts="v")
        prefetch_xt(4)
        tile_unit(1, parts="v")
        prefetch_xt(5)
        for t in range(2):
            attn(0, 0, t, reserve=6)
            attn(0, 1, t, reserve=6)
            pend += [(0, 4 * t + i, et) for i in range(4) for et in range(D // 512)]
        for t in range(2, NTT // 2):
            tile_unit(t)
            prefetch_xt(t + 4)
            attn(0, 0, t, reserve=6)
            attn(0, 1, t, reserve=6)
            pend += [(0, 4 * t + i, et) for i in range(4) for et in range(D // 512)]
        for t in range(NTT // 2, NTT):
            tile_unit(t)
            emit_pend(4)
        # a standing reserve of units keeps the PE fed through every
        # ones-mm -> Ln -> Exp -> mult boundary chain in the b1 phase
        for qt in range(S // QT):
            attn(1, 0, qt, reserve=12)
            emit_pend(2)
            attn(1, 1, qt, reserve=12)
            emit_pend(2)
            pend += [(1, 4 * qt + i, et) for i in range(4) for et in range(D // 512)]
        emit_pend(len(pend))
    return _patch_serialization(nc)


def _prep_inputs(x, wq, wk, wv, wo, freqs_cos, freqs_sin):
    import ml_dtypes

    bf16 = ml_dtypes.bfloat16
    perm = np.concatenate([np.arange(0, HD, 2), np.arange(1, HD, 2)])

    xt = np.ascontiguousarray(x.reshape(BS, D).T)          # [D, BS]
    # tile-major pack: [128, NTT, NKT*TT] so a multi-chunk DMA piece of one
    # token tile is contiguous per partition (4-16KB DMA elements)
    xt_r = np.ascontiguousarray(
        xt.reshape(NKT, 128, NTT, TT).transpose(1, 2, 0, 3).reshape(
            128, NTT, NKT * TT
        )
    ).astype(bf16)

    cosT = freqs_cos.T.astype(np.float32)                  # [64, S]
    sinT = freqs_sin.T.astype(np.float32)
    cs = np.concatenate(
        [np.tile(cosT, (1, B)), np.tile(sinT, (1, B))], axis=0
    ).astype(bf16)                                         # [128, BS]

    i = np.arange(KC)[:, None]
    m4 = np.concatenate(
        [(i <= np.arange(w)[None, :]).astype(np.float32) for w in (512, 384, 256, 128)],
        axis=1,
    ).astype(bf16)                                         # [128, 1280] packed triangles

    def pack_w(wmat_cols):
        # wmat_cols: [D, 2*HD] -> [128, NKT * 2*HD] (flat chunk-major)
        return np.ascontiguousarray(
            wmat_cols.reshape(NKT, 128, 2 * HD).transpose(1, 0, 2).reshape(
                128, NKT * 2 * HD
            )
        ).astype(bf16)

    in_maps = []
    for c in range(NCORES):
        heads = [HPC * c + hh for hh in range(HPC)]
        wq_c = np.concatenate(
            [wq[h * HD : (h + 1) * HD][perm].T for h in heads], axis=1
        )                                                  # [D, 2*HD]
        wk_c = np.concatenate(
            [wk[h * HD : (h + 1) * HD][perm].T for h in heads], axis=1
        )
        wv_c = np.concatenate(
            [wv[h * HD : (h + 1) * HD].T for h in heads], axis=1
        )
        wo_c = np.stack(
            [wo[:, h * HD : (h + 1) * HD].T for h in heads], axis=0
        )                                                  # [2, HD, D]
        wo_r = np.ascontiguousarray(wo_c.transpose(1, 0, 2)).astype(bf16).reshape(
            128, 2 * D
        )
        in_maps.append(
            dict(
                xt=xt_r,
                wq=pack_w(wq_c),
                wk=pack_w(wk_c),
                wv=pack_w(wv_c),
                wo=wo_r,
                cs=cs,
                m4=m4,
            )
        )
    return in_maps


_NC_CACHE = {}


def kernel(x, wq, wk, wv, wo, freqs_cos, freqs_sin, mask):
    from concourse.bass_utils import run_bass_kernel_spmd

    in_maps = _prep_inputs(x, wq, wk, wv, wo, freqs_cos, freqs_sin)
    if "nc" not in _NC_CACHE:
        _NC_CACHE["nc"] = _build_nc()
    nc = _NC_CACHE["nc"]
    res = run_bass_kernel_spmd(nc, in_maps, core_ids=list(range(NCORES)))
    parts = [r["out"].astype(np.float32) for r in res.results]
    out = np.sum(np.stack(parts, 0), axis=0, dtype=np.float32)
    return out.reshape(B, S, D)



# revision 44
# speedup vs baseline: 1.0316x; 1.0027x over previous
"""Distributed Trainium2 kernel for 16-head causal attention with RoPE.

B=2, S=2048, D=2048, H=16, HD=128. Tensor-parallel over heads: core c owns
heads {2c, 2c+1}. Each core computes q/k/v projections for its heads,
RoPE, causal attention, and a partial output projection (wo row-shard);
the host sums the 8 partials (the unshard step for a row-sharded wo).

Device-side layout choices (all transposes are done on the host):
  - x is fed pre-transposed as xt[d, tok] so every matmul contracts over
    the partition axis with no on-device transposes.
  - q/k are produced head-dim-major (qT[hd, tok]); the RoPE even/odd pair
    permutation is folded into the wq/wk columns on the host, so RoPE is
    six plain elementwise ops on [64, tok] slices.
  - scores are computed transposed (scoresT[k, q]) in 128-wide k chunks,
    software-pipelined 3 deep so the PE never waits on the exp chain.
    The diagonal 128-blocks are triangularly trimmed (q range restricted
    per block). Softmax sums over k accumulate on DVE (bf16 adds), one
    ones-matmul per (b,h,qt) reduces+broadcasts, and a DVE divide
    normalizes (no Ln/Exp round-trip on the ACT engine).
  - attention output oT[hd, q] is exactly the lhsT the output projection
    needs, so the whole pipeline has zero on-device transposes.
  - out partials are cast to fp16, staged [128, 2048] per token block,
    and stored with one DMA per block (host sums partials in fp32).
"""

import numpy as np
from contextlib import ExitStack

B, S, D = 2, 2048, 2048
H, HD, HALF = 16, 128, 64
BS = B * S
NCORES = 8
HPC = H // NCORES          # heads per core
TT = 512                   # token tile for projections
QT = 512                   # q tile in attention
KC = 128                   # k chunk in attention
NKT = D // 128             # 16 contraction chunks of the model dim
NTT = BS // TT             # 8 token tiles
ISQRT = 1.0 / float(np.sqrt(HD))


def _legalize_waits(bir: bytes) -> bytes:
    """Split multi-wait sync_info into standalone EventSemaphore instructions.

    The neuronxcc walrus codegen only encodes ONE sync wait slot on compute
    instructions (Matmult/TensorTensor/...); Tile's sem-assignment freely
    emits several. Hoisting the extras into same-engine EventSemaphore
    instructions placed immediately before the consumer is semantically
    identical (the sequencer blocks on them in program order).
    """
    import json

    d = json.loads(bir)
    wid = 0
    for fn in d["functions"]:
        for blk in fn["blocks"]:
            out = []
            for inst in blk["instructions"]:
                si = inst.get("sync_info")
                if si:
                    waits = si.get("on_wait") or []
                    if len(waits) > 1 and inst.get("engine") not in (None, "Unassigned"):
                        for w in waits[:-1]:
                            wid += 1
                            out.append(
                                {
                                    "debug": inst.get("debug", 0),
                                    "engine": inst["engine"],
                                    "ins": [],
                                    "name": f"hoisted-wait-{wid}",
                                    "opcode": "EventSemaphore",
                                    "outs": [],
                                    "sync_info": {"on_update": [], "on_wait": [w]},
                                }
                            )
                        si["on_wait"] = [waits[-1]]
                out.append(inst)
            blk["instructions"] = out
    return json.dumps(d).encode()


def _patch_serialization(nc):
    import types

    orig = nc.to_json_bytes

    def patched(self):
        return _legalize_waits(orig())

    nc.to_json_bytes = types.MethodType(patched, nc)
    return nc


def _build_nc():
    import concourse.bass as bass
    import concourse.tile as tile
    from concourse import mybir

    f32 = mybir.dt.float32
    f16 = mybir.dt.float16
    bf16 = mybir.dt.bfloat16
    Exp = mybir.ActivationFunctionType.Exp
    Ln = mybir.ActivationFunctionType.Ln
    mult = mybir.AluOpType.mult
    sub = mybir.AluOpType.subtract
    add = mybir.AluOpType.add

    nc = bass.Bass()

    # All weight/activation DRAM params are FLAT 2D (or tile-major 3D for
    # xt) so multi-chunk DMA pieces lower to 4-16KB contiguous elements.
    # The SDMA rings drain ~100GB/s at 1KB elements but ~300GB/s at 4KB+,
    # and the whole early phase is delivery-bound.
    WCOLS = NKT * 2 * HD
    XCOLS = NKT * TT
    xt_h = nc.declare_dram_parameter("xt", [128, NTT, XCOLS], bf16, isOutput=False)
    wq_h = nc.declare_dram_parameter("wq", [128, WCOLS], bf16, isOutput=False)
    wk_h = nc.declare_dram_parameter("wk", [128, WCOLS], bf16, isOutput=False)
    wv_h = nc.declare_dram_parameter("wv", [128, WCOLS], bf16, isOutput=False)
    wo_h = nc.declare_dram_parameter("wo", [128, 2 * D], bf16, isOutput=False)
    cs_h = nc.declare_dram_parameter("cs", [128, BS], bf16, isOutput=False)
    m4_h = nc.declare_dram_parameter("m4", [128, 1280], bf16, isOutput=False)
    out_h = nc.declare_dram_parameter("out", [BS, D], f16, isOutput=True)

    with ExitStack() as ctx:
        tc = ctx.enter_context(tile.TileContext(nc))
        const = ctx.enter_context(tc.tile_pool(name="const", bufs=1))
        persist = ctx.enter_context(tc.tile_pool(name="persist", bufs=1))
        xtp = ctx.enter_context(tc.tile_pool(name="xtp", bufs=4))
        expp = ctx.enter_context(tc.tile_pool(name="expp", bufs=4))
        esp = ctx.enter_context(tc.tile_pool(name="esp", bufs=3))
        ropet = ctx.enter_context(tc.tile_pool(name="ropet", bufs=6))
        # outp 3-deep: copies must not back up on out-store DMA completion
        # (a blocked copy sits in the ACT stream ahead of attention exps)
        outp = ctx.enter_context(tc.tile_pool(name="outp", bufs=3))
        psA = ctx.enter_context(tc.tile_pool(name="psA", bufs=2, space="PSUM"))
        psS = ctx.enter_context(tc.tile_pool(name="psS", bufs=2, space="PSUM"))
        # ov double-buffered: the next attention's PV accumulation must not
        # wait for the previous one's normalize chain to release the bank.
        # sm shares psA's slots (it lives ~1.4us, between ones-mm and Ln).
        psO = ctx.enter_context(tc.tile_pool(name="psO", bufs=2, space="PSUM"))

        # ---- constants into SBUF ----
        # Flat 2D tiles mirror the flat DRAM params; matmul operands slice
        # computed column ranges.
        wq_sb = const.tile([128, WCOLS], bf16, tag="wq")
        wk_sb = const.tile([128, WCOLS], bf16, tag="wk")
        wv_sb = const.tile([128, WCOLS], bf16, tag="wv")
        wo_sb = const.tile([128, 2 * D], bf16, tag="wo")
        cs_sb = const.tile([128, BS], bf16, tag="cs")
        m4_sb = const.tile([128, 1280], bf16, tag="m4")
        ones_sb = const.tile([128, 128], bf16, tag="ones")
        # Startup loads are queued deep on both HWDGE rings immediately in
        # need order (the SDMA engines pipeline across queued DMAs, so a
        # deep queue drains faster than issue-as-needed). Sync carries the
        # PE-critical wq + xt stream; scalar carries the rest of the
        # weights, then xt tiles 2-3.
        # Early compute order is q(t0), q(t1), k(t0), k(t1), v(t0), v(t1):
        # only wq/xt0/xt1 gate the first ~25us.
        C4 = 4 * 2 * HD  # 4 contraction chunks of a weight matrix, flat
        X4 = 4 * TT      # 4 contraction chunks of an xt tile, flat
        # Three parallel DMA streams (per-ring early throughput is only
        # ~200GB/s, so the 7MB the first 35us of compute needs must be
        # split):
        #   sync (HWDGE):   xt0, xt1 head   -> later all out stores
        #   scalar (HWDGE): wq, cs0, wk, xt1 tail, wv (the weight path)
        #   gpsimd (SWDGE): bulk with slack: m4, cs1, xt2, xt3, wo, xt4-7
        xt_tiles = [
            xtp.tile([128, XCOLS], bf16, tag="xt", name=f"xt{i}") for i in range(4)
        ]
        nc.sync.dma_start(wq_sb[:, 0:C4], wq_h[:, 0:C4])
        nc.sync.dma_start(xt_tiles[0][:, 0:X4], xt_h[:, 0, 0:X4])
        nc.sync.dma_start(wq_sb[:, C4:WCOLS], wq_h[:, C4:WCOLS])
        for p in range(1, 4):
            nc.sync.dma_start(
                xt_tiles[0][:, p * X4 : (p + 1) * X4],
                xt_h[:, 0, p * X4 : (p + 1) * X4],
            )
        for p in range(2):
            nc.sync.dma_start(
                xt_tiles[1][:, p * X4 : (p + 1) * X4],
                xt_h[:, 1, p * X4 : (p + 1) * X4],
            )
        nc.scalar.dma_start(cs_sb[:, 0 : 2 * TT], cs_h[:, 0 : 2 * TT])
        nc.scalar.dma_start(wk_sb[:, 0:C4], wk_h[:, 0:C4])
        nc.scalar.dma_start(wk_sb[:, C4:WCOLS], wk_h[:, C4:WCOLS])
        for p in range(2, 4):
            nc.scalar.dma_start(
                xt_tiles[1][:, p * X4 : (p + 1) * X4],
                xt_h[:, 1, p * X4 : (p + 1) * X4],
            )
        nc.scalar.dma_start(wv_sb[:], wv_h[:])
        nc.scalar.dma_start(m4_sb[:], m4_h[:])
        nc.scalar.dma_start(cs_sb[:, 2 * TT :], cs_h[:, 2 * TT :])
        for t in (2, 3):
            for p in range(2):
                nc.scalar.dma_start(
                    xt_tiles[t][:, p * 2 * X4 : (p + 1) * 2 * X4],
                    xt_h[:, t, p * 2 * X4 : (p + 1) * 2 * X4],
                )
        nc.scalar.dma_start(wo_sb[:], wo_h[:])

        # tiles 4-7 prefetch on scalar, each emitted right after the tile
        # unit that frees its pool slot (so the slot wait is already
        # satisfied and never blocks the ACT stream), keeping the sync ring
        # free for out stores. (SWDGE/gpsimd rings measurably slow the
        # HWDGE rings when used concurrently — keep everything on HWDGE.)
        def prefetch_xt(t):
            xt_t = xtp.tile([128, XCOLS], bf16, tag="xt", name=f"xt{t}")
            assert len(xt_tiles) == t
            xt_tiles.append(xt_t)
            for p in range(2):
                nc.scalar.dma_start(
                    xt_t[:, p * 2 * X4 : (p + 1) * 2 * X4],
                    xt_h[:, t, p * 2 * X4 : (p + 1) * 2 * X4],
                )

        nc.vector.memset(ones_sb[:], 1.0)

        # PE warm-up: the HAM clock gate needs ~3us of sustained matmul
        # activity to lift the PE from 1.2 to 2.4 GHz. The first real
        # matmuls can't start until their DMAs land (~11us), so burn the
        # wait warming the array on the memset ones tile (no DMA deps).
        warm_ps = psS.tile([128, QT], mybir.dt.float32, tag="sc")
        for i in range(16):
            nc.tensor.matmul(
                warm_ps[:, (i % 4) * 128 : (i % 4) * 128 + 128],
                ones_sb[:], ones_sb[:], start=True, stop=True,
            )
        # consume so DCE keeps the warm-up and the PSUM slot is released
        warm_sink = const.tile([1, 8], mybir.dt.float32, tag="wsink")
        nc.scalar.copy(warm_sink[0:1, 0:2], warm_ps[0:1, 0:2])

        # DVE pre-touch of DMA-written constants: TensorTensor instructions
        # encode only one sync-wait slot, so the DVE vector clock must have
        # observed these DMAs before any TT reads them (else walrus dies with
        # "Too many sync wait commands").
        scratch = const.tile([1, 8], bf16, tag="scratch")
        nc.vector.tensor_copy(scratch[0:1, 0:2], cs_sb[0:1, 0:2])
        nc.vector.tensor_copy(scratch[0:1, 2:4], m4_sb[0:1, 0:2])

        # persistent activations
        qr = persist.tile([128, HPC, BS], bf16, tag="qr")   # rotated qT per head
        kr = persist.tile([128, HPC, BS], bf16, tag="kr")   # rotated kT per head
        v_sb = persist.tile([128, BS // 128, 2 * HD], bf16, tag="v")  # tok-major v
        on_sb = persist.tile([128, HPC, B, S], bf16, tag="on")  # normalized oT

        # ---- phase 1 unit: one token tile's q/k/v projections + RoPE ----
        alt = [0]

        def tile_unit(t, parts="qkv"):
            t0 = t * TT
            xt_t = xt_tiles[t]

            # During tiles 0-3 the score pool psS is idle: alternate the
            # projection PSUM between psA and psS so a slow RoPE drain
            # (DVE, gated on the cs DMA) can lag 4 groups behind the PE
            # without starving it of accumulator slots.
            def proj_tile():
                alt[0] += 1
                if alt[0] % 2:
                    return psS.tile([128, TT], mybir.dt.float32, tag="sc", name="pp")
                return psA.tile([128, TT], mybir.dt.float32, tag="proj", name="pp")

            co = cs_sb[0:HALF, t0 : t0 + TT]
            si = cs_sb[HALF:128, t0 : t0 + TT]
            # q for both heads first (only needs wq), then k for both
            # heads: gives the wk DMA (behind wq on sync) time to land
            # without stalling the PE on tile 0.
            wlist = []
            if "q" in parts:
                wlist.append((wq_sb, qr))
            if "k" in parts:
                wlist.append((wk_sb, kr))
            for w_sb, dstT in wlist:
                for h in range(HPC):
                    pq = proj_tile()
                    for c in range(NKT):
                        wc = c * 2 * HD + h * HD
                        nc.tensor.matmul(
                            pq[:],
                            w_sb[:, wc : wc + HD],
                            xt_t[:, c * TT : (c + 1) * TT],
                            start=(c == 0),
                            stop=(c == NKT - 1),
                        )
                    t1 = ropet.tile([HALF, TT], bf16, tag="rt")
                    t2 = ropet.tile([HALF, TT], bf16, tag="rt")
                    t3 = ropet.tile([HALF, TT], bf16, tag="rt")
                    t4 = ropet.tile([HALF, TT], bf16, tag="rt")
                    # all four PSUM-reading muls first so pq's slot frees
                    # as soon as possible (the combines only read SBUF)
                    nc.vector.tensor_tensor(t1[:], pq[0:HALF, :], co, mult)
                    nc.vector.tensor_tensor(t2[:], pq[HALF:128, :], si, mult)
                    nc.vector.tensor_tensor(t3[:], pq[0:HALF, :], si, mult)
                    nc.vector.tensor_tensor(t4[:], pq[HALF:128, :], co, mult)
                    # combines are SBUF-only and far off the critical path
                    # (consumed by attention much later): run them on the
                    # otherwise-idle Pool engine to unload DVE
                    nc.gpsimd.tensor_tensor(
                        dstT[0:HALF, h, t0 : t0 + TT], t1[:], t2[:], sub
                    )
                    nc.gpsimd.tensor_tensor(
                        dstT[HALF:128, h, t0 : t0 + TT], t3[:], t4[:], add
                    )

            # v projection, token-major [tok, 2*HD]
            for m in range(TT // 128 if "v" in parts else 0):
                pv = proj_tile()
                for c in range(NKT):
                    nc.tensor.matmul(
                        pv[:, 0 : 2 * HD],
                        xt_t[:, c * TT + m * 128 : c * TT + (m + 1) * 128],
                        wv_sb[:, c * 2 * HD : (c + 1) * 2 * HD],
                        start=(c == 0),
                        stop=(c == NKT - 1),
                    )
                g = t * (TT // 128) + m
                nc.scalar.copy(v_sb[:, g, :], pv[:, 0 : 2 * HD])

        # out-proj work queue: single [128,512] units threaded into the
        # attention chunk loop so the PE stream never has copy-gated bursts
        pend = []

        def emit_pend(n):
            for _ in range(min(n, len(pend))):
                outproj_one(*pend.pop(0))

        def attn(b, h, qt, reserve=0):
            q0 = b * S + qt * QT
            # k chunks packed two per PSUM tile / exp call (halves the ACT
            # instruction count and its per-call PSUM access penalty).
            # Full-width unmasked pairs below the diagonal first, then the
            # two triangularly-trimmed masked diagonal pairs, whose chunks
            # pack at column offsets c0 matching the m4 mask layout.
            pairs = [
                ([(2 * p, 0, QT, 0), (2 * p + 1, 0, QT, QT)], None)
                for p in range(2 * qt)
            ]
            pairs.append(
                ([(4 * qt, 0, QT, 0), (4 * qt + 1, 128, QT - 128, QT)], 0)
            )
            pairs.append(
                ([(4 * qt + 2, 256, QT - 256, 0),
                  (4 * qt + 3, 384, QT - 384, QT - 256)], 896)
            )
            n = 4 * qt + 4
            ov = psO.tile([128, QT], mybir.dt.float32, tag="ov")
            es = esp.tile([128, QT], bf16, tag="es")
            meta = []
            flat = [0]

            def emit_pair(pcs, moff):
                wtot = pcs[-1][3] + pcs[-1][2]
                sc = psS.tile([128, 2 * QT], mybir.dt.float32, tag="sc", name="sc")
                for kb, qoff, w, c0 in pcs:
                    k0 = b * S + kb * KC
                    nc.tensor.matmul(
                        sc[:, c0 : c0 + w],
                        kr[:, h, k0 : k0 + KC],
                        qr[:, h, q0 + qoff : q0 + QT],
                        start=True,
                        stop=True,
                    )
                e = expp.tile([128, 2 * QT], bf16, tag="e", name="e")
                nc.scalar.activation(e[:, 0:wtot], sc[:, 0:wtot], Exp, scale=ISQRT)
                if moff is not None:
                    nc.vector.tensor_tensor(
                        e[:, 0:wtot], e[:, 0:wtot], m4_sb[:, moff : moff + wtot], mult
                    )
                ms = []
                for kb, qoff, w, c0 in pcs:
                    if flat[0] == 0:
                        nc.vector.tensor_copy(es[:], e[:, 0:QT])
                    else:
                        nc.vector.tensor_tensor(
                            es[:, qoff:QT], es[:, qoff:QT], e[:, c0 : c0 + w], add
                        )
                    ms.append((kb, qoff, w, c0, flat[0]))
                    flat[0] += 1
                meta.append((e, ms))

            def do_pv(pi):
                e, ms = meta[pi]
                for kb, qoff, w, c0, ci in ms:
                    gk = (b * S + kb * KC) // 128
                    nc.tensor.matmul(
                        ov[:, qoff:QT],
                        v_sb[:, gk, h * HD : (h + 1) * HD],
                        e[:, c0 : c0 + w],
                        start=(ci == 0),
                        stop=(ci == n - 1),
                        skip_group_check=True,
                    )

            for pi, (pcs, moff) in enumerate(pairs):
                emit_pair(pcs, moff)
                if pi >= 1:
                    do_pv(pi - 1)
                if len(pend) > reserve:
                    emit_pend(min(2, len(pend) - reserve))
            do_pv(len(pairs) - 1)
            # sum over k (partition axis) via ones-matmul: reduces and
            # broadcasts the denominator to all 128 partitions in one shot
            sm = psA.tile([128, QT], mybir.dt.float32, tag="proj", name="sm")
            nc.tensor.matmul(sm[:], ones_sb[:], es[:], start=True, stop=True)
            # fill the serial ones-mm -> Ln -> Exp -> mult latency with any
            # queued out-proj units (they depend only on earlier attns)
            emit_pend(4)
            # 1/sum via exp(-ln(sum)): Ln and Exp share one ACT table so no
            # table reloads (DVE has no divide, and TT can't read 2x PSUM).
            lnt = esp.tile([128, QT], mybir.dt.float32, tag="smsb")
            nc.scalar.activation(lnt[:], sm[:], Ln)
            rr = esp.tile([128, QT], mybir.dt.float32, tag="smsb")
            nc.scalar.activation(rr[:], lnt[:], Exp, scale=-1.0)
            # pre-touch rr on DVE so the norm TT only waits on PE
            nc.vector.tensor_copy(scratch[0:1, 4:6], rr[0:1, 0:2])
            nc.vector.tensor_tensor(
                on_sb[:, h, b, qt * QT : qt * QT + QT], ov[:], rr[:], mult
            )

        ecount = 0
        cur_stage = [None]
        # During the tiles 4-7 stretch ACT has no exp work but DVE carries
        # all the RoPE muls; copies landing on DVE there queue ahead of the
        # muls that must drain the projection PSUM, stalling PE group
        # starts. Force copies to ACT in that phase.
        force_copy = [None]

        def outproj_one(b, tcn, et):
            nonlocal ecount
            po = psA.tile([128, 512], mybir.dt.float32, tag="proj")
            for j in range(HPC):
                nc.tensor.matmul(
                    po[:],
                    on_sb[:, j, b, tcn * 128 : tcn * 128 + 128],
                    wo_sb[:, j * D + et * 512 : j * D + et * 512 + 512],
                    start=(j == 0),
                    stop=(j == HPC - 1),
                )
            if et == 0:
                cur_stage[0] = outp.tile([128, D], f16, tag="ob", name="ob")
            ob = cur_stage[0]
            if force_copy[0] is not None:
                eng = force_copy[0]
            else:
                eng = (nc.scalar.copy, nc.vector.tensor_copy)[ecount % 2]
            eng(ob[:, et * 512 : et * 512 + 512], po[:])
            ecount += 1
            rows = slice(b * S + tcn * 128, b * S + tcn * 128 + 128)
            if b == 1 and tcn >= 12:
                # tail-critical blocks: store per half; the final block
                # per-unit so the very last transfer is only 128KB
                if tcn == 15:
                    cols = slice(et * 512, et * 512 + 512)
                    nc.sync.dma_start(out_h[rows, cols], ob[:, cols])
                elif et == 1 or et == 3:
                    cols = slice(0, 1024) if et == 1 else slice(1024, 2048)
                    nc.sync.dma_start(out_h[rows, cols], ob[:, cols])
            elif et == 3:
                # one full-width store per token block: 4KB contiguous rows
                # drain the ring at roughly twice the rate of 2KB halves
                nc.sync.dma_start(out_h[rows, :], ob[:, :])

        # Batch-0 token tiles first; then batch-1 tiles with batch-0's
        # attention interleaved between them (its exp load lands where
        # ScalarE is otherwise idle, and it needs no PSUM "proj" slots so
        # it doesn't fight the projection groups). Out-proj units run as
        # single [128,512] groups threaded into the batch-1 attention
        # chunk loops (emit_pend inside attn), never in bursts.
        # Batch-0 attention q-tile t only needs tokens [0, (t+1)*512) of
        # q/k/v — run it right after token tile t, where its matmuls and
        # exps fill the startup DMA-delivery stalls. Tiles 4-7 (batch 1
        # projections) then run as one PE-saturated stretch, draining
        # queued out-proj units into any gaps.
        # tiles 0-1 run q(t0), k(t0), q(t1), k(t1): tile 0's work only
        # needs wq/wk/xt0 (first on both rings), so the PE never idles on
        # xt1's tail the way q(t0),q(t1) back-to-back would
        tile_unit(0, parts="q")
        tile_unit(0, parts="k")
        tile_unit(1, parts="q")
        tile_unit(1, parts="k")
        tile_unit(0, parts="v")
        prefetch_xt(4)
        tile_unit(1, parts="v")
        prefetch_xt(5)
        for t in range(2):
            attn(0, 0, t, reserve=6)
            attn(0, 1, t, reserve=6)
            pend += [(0, 4 * t + i, et) for i in range(4) for et in range(D // 512)]
        for t in range(2, NTT // 2):
            tile_unit(t)
            prefetch_xt(t + 4)
            attn(0, 0, t, reserve=6)
            attn(0, 1, t, reserve=6)
            pend += [(0, 4 * t + i, et) for i in range(4) for et in range(D // 512)]
        force_copy[0] = nc.scalar.copy
        for t in range(NTT // 2, NTT):
            tile_unit(t)
            emit_pend(4)
        force_copy[0] = None
        # a standing reserve of units keeps the PE fed through every
        # ones-mm -> Ln -> Exp -> mult boundary chain in the b1 phase
        for qt in range(S // QT):
            attn(1, 0, qt, reserve=12)
            emit_pend(2)
            attn(1, 1, qt, reserve=12)
            emit_pend(2)
            pend += [(1, 4 * qt + i, et) for i in range(4) for et in range(D // 512)]
        emit_pend(len(pend))
    return _patch_serialization(nc)


def _prep_inputs(x, wq, wk, wv, wo, freqs_cos, freqs_sin):
    import ml_dtypes

    bf16 = ml_dtypes.bfloat16
    perm = np.concatenate([np.arange(0, HD, 2), np.arange(1, HD, 2)])

    xt = np.ascontiguousarray(x.reshape(BS, D).T)          # [D, BS]
    # tile-major pack: [128, NTT, NKT*TT] so a multi-chunk DMA piece of one
    # token tile is contiguous per partition (4-16KB DMA elements)
    xt_r = np.ascontiguousarray(
        xt.reshape(NKT, 128, NTT, TT).transpose(1, 2, 0, 3).reshape(
            128, NTT, NKT * TT
        )
    ).astype(bf16)

    cosT = freqs_cos.T.astype(np.float32)                  # [64, S]
    sinT = freqs_sin.T.astype(np.float32)
    cs = np.concatenate(
        [np.tile(cosT, (1, B)), np.tile(sinT, (1, B))], axis=0
    ).astype(bf16)                                         # [128, BS]

    i = np.arange(KC)[:, None]
    m4 = np.concatenate(
        [(i <= np.arange(w)[None, :]).astype(np.float32) for w in (512, 384, 256, 128)],
        axis=1,
    ).astype(bf16)                                         # [128, 1280] packed triangles

    def pack_w(wmat_cols):
        # wmat_cols: [D, 2*HD] -> [128, NKT * 2*HD] (flat chunk-major)
        return np.ascontiguousarray(
            wmat_cols.reshape(NKT, 128, 2 * HD).transpose(1, 0, 2).reshape(
                128, NKT * 2 * HD
            )
        ).astype(bf16)

    in_maps = []
    for c in range(NCORES):
        heads = [HPC * c + hh for hh in range(HPC)]
        wq_c = np.concatenate(
            [wq[h * HD : (h + 1) * HD][perm].T for h in heads], axis=1
        )                                                  # [D, 2*HD]
        wk_c = np.concatenate(
            [wk[h * HD : (h + 1) * HD][perm].T for h in heads], axis=1
        )
        wv_c = np.concatenate(
            [wv[h * HD : (h + 1) * HD].T for h in heads], axis=1
        )
        wo_c = np.stack(
            [wo[:, h * HD : (h + 1) * HD].T for h in heads], axis=0
        )                                                  # [2, HD, D]
        wo_r = np.ascontiguousarray(wo_c.transpose(1, 0, 2)).astype(bf16).reshape(
            128, 2 * D
        )
        in_maps.append(
            dict(
                xt=xt_r,
                wq=pack_w(wq_c),
                wk=pack_w(wk_c),
                wv=pack_w(wv_c),
                wo=wo_r,
                cs=cs,
                m4=m4,
            )
        )
    return in_maps


_NC_CACHE = {}


def kernel(x, wq, wk, wv, wo, freqs_cos, freqs_sin, mask):
    from concourse.bass_utils import run_bass_kernel_spmd

    in_maps = _prep_inputs(x, wq, wk, wv, wo, freqs_cos, freqs_sin)
    if "nc" not in _NC_CACHE:
        _NC_CACHE["nc"] = _build_nc()
    nc = _NC_CACHE["nc"]
    res = run_bass_kernel_spmd(nc, in_maps, core_ids=list(range(NCORES)))
    parts = [r["out"].astype(np.float32) for r in res.results]
    out = np.sum(np.stack(parts, 0), axis=0, dtype=np.float32)
    return out.reshape(B, S, D)



# revision 45
# speedup vs baseline: 1.0370x; 1.0052x over previous
"""Distributed Trainium2 kernel for 16-head causal attention with RoPE.

B=2, S=2048, D=2048, H=16, HD=128. Tensor-parallel over heads: core c owns
heads {2c, 2c+1}. Each core computes q/k/v projections for its heads,
RoPE, causal attention, and a partial output projection (wo row-shard);
the host sums the 8 partials (the unshard step for a row-sharded wo).

Device-side layout choices (all transposes are done on the host):
  - x is fed pre-transposed as xt[d, tok] so every matmul contracts over
    the partition axis with no on-device transposes.
  - q/k are produced head-dim-major (qT[hd, tok]); the RoPE even/odd pair
    permutation is folded into the wq/wk columns on the host, so RoPE is
    six plain elementwise ops on [64, tok] slices.
  - scores are computed transposed (scoresT[k, q]) in 128-wide k chunks,
    software-pipelined 3 deep so the PE never waits on the exp chain.
    The diagonal 128-blocks are triangularly trimmed (q range restricted
    per block). Softmax sums over k accumulate on DVE (bf16 adds), one
    ones-matmul per (b,h,qt) reduces+broadcasts, and a DVE divide
    normalizes (no Ln/Exp round-trip on the ACT engine).
  - attention output oT[hd, q] is exactly the lhsT the output projection
    needs, so the whole pipeline has zero on-device transposes.
  - out partials are cast to fp16, staged [128, 2048] per token block,
    and stored with one DMA per block (host sums partials in fp32).
"""

import numpy as np
from contextlib import ExitStack

B, S, D = 2, 2048, 2048
H, HD, HALF = 16, 128, 64
BS = B * S
NCORES = 8
HPC = H // NCORES          # heads per core
TT = 512                   # token tile for projections
QT = 512                   # q tile in attention
KC = 128                   # k chunk in attention
NKT = D // 128             # 16 contraction chunks of the model dim
NTT = BS // TT             # 8 token tiles
ISQRT = 1.0 / float(np.sqrt(HD))


def _legalize_waits(bir: bytes) -> bytes:
    """Split multi-wait sync_info into standalone EventSemaphore instructions.

    The neuronxcc walrus codegen only encodes ONE sync wait slot on compute
    instructions (Matmult/TensorTensor/...); Tile's sem-assignment freely
    emits several. Hoisting the extras into same-engine EventSemaphore
    instructions placed immediately before the consumer is semantically
    identical (the sequencer blocks on them in program order).
    """
    import json

    d = json.loads(bir)
    wid = 0
    for fn in d["functions"]:
        for blk in fn["blocks"]:
            out = []
            for inst in blk["instructions"]:
                si = inst.get("sync_info")
                if si:
                    waits = si.get("on_wait") or []
                    if len(waits) > 1 and inst.get("engine") not in (None, "Unassigned"):
                        for w in waits[:-1]:
                            wid += 1
                            out.append(
                                {
                                    "debug": inst.get("debug", 0),
                                    "engine": inst["engine"],
                                    "ins": [],
                                    "name": f"hoisted-wait-{wid}",
                                    "opcode": "EventSemaphore",
                                    "outs": [],
                                    "sync_info": {"on_update": [], "on_wait": [w]},
                                }
                            )
                        si["on_wait"] = [waits[-1]]
                out.append(inst)
            blk["instructions"] = out
    return json.dumps(d).encode()


def _patch_serialization(nc):
    import types

    orig = nc.to_json_bytes

    def patched(self):
        return _legalize_waits(orig())

    nc.to_json_bytes = types.MethodType(patched, nc)
    return nc


def _build_nc():
    import concourse.bass as bass
    import concourse.tile as tile
    from concourse import mybir

    f32 = mybir.dt.float32
    f16 = mybir.dt.float16
    bf16 = mybir.dt.bfloat16
    Exp = mybir.ActivationFunctionType.Exp
    Ln = mybir.ActivationFunctionType.Ln
    mult = mybir.AluOpType.mult
    sub = mybir.AluOpType.subtract
    add = mybir.AluOpType.add

    nc = bass.Bass()

    # All weight/activation DRAM params are FLAT 2D (or tile-major 3D for
    # xt) so multi-chunk DMA pieces lower to 4-16KB contiguous elements.
    # The SDMA rings drain ~100GB/s at 1KB elements but ~300GB/s at 4KB+,
    # and the whole early phase is delivery-bound.
    WCOLS = NKT * 2 * HD
    XCOLS = NKT * TT
    xt_h = nc.declare_dram_parameter("xt", [128, NTT, XCOLS], bf16, isOutput=False)
    wq_h = nc.declare_dram_parameter("wq", [128, WCOLS], bf16, isOutput=False)
    wk_h = nc.declare_dram_parameter("wk", [128, WCOLS], bf16, isOutput=False)
    wv_h = nc.declare_dram_parameter("wv", [128, WCOLS], bf16, isOutput=False)
    wo_h = nc.declare_dram_parameter("wo", [128, 2 * D], bf16, isOutput=False)
    cs_h = nc.declare_dram_parameter("cs", [128, BS], bf16, isOutput=False)
    m4_h = nc.declare_dram_parameter("m4", [128, 1280], bf16, isOutput=False)
    out_h = nc.declare_dram_parameter("out", [BS, D], f16, isOutput=True)

    with ExitStack() as ctx:
        tc = ctx.enter_context(tile.TileContext(nc))
        const = ctx.enter_context(tc.tile_pool(name="const", bufs=1))
        persist = ctx.enter_context(tc.tile_pool(name="persist", bufs=1))
        xtp = ctx.enter_context(tc.tile_pool(name="xtp", bufs=4))
        expp = ctx.enter_context(tc.tile_pool(name="expp", bufs=4))
        esp = ctx.enter_context(tc.tile_pool(name="esp", bufs=3))
        ropet = ctx.enter_context(tc.tile_pool(name="ropet", bufs=6))
        # outp 3-deep: copies must not back up on out-store DMA completion
        # (a blocked copy sits in the ACT stream ahead of attention exps)
        outp = ctx.enter_context(tc.tile_pool(name="outp", bufs=3))
        psA = ctx.enter_context(tc.tile_pool(name="psA", bufs=2, space="PSUM"))
        psS = ctx.enter_context(tc.tile_pool(name="psS", bufs=2, space="PSUM"))
        # ov double-buffered: the next attention's PV accumulation must not
        # wait for the previous one's normalize chain to release the bank.
        # sm shares psA's slots (it lives ~1.4us, between ones-mm and Ln).
        psO = ctx.enter_context(tc.tile_pool(name="psO", bufs=2, space="PSUM"))

        # ---- constants into SBUF ----
        # Flat 2D tiles mirror the flat DRAM params; matmul operands slice
        # computed column ranges.
        wq_sb = const.tile([128, WCOLS], bf16, tag="wq")
        wk_sb = const.tile([128, WCOLS], bf16, tag="wk")
        wv_sb = const.tile([128, WCOLS], bf16, tag="wv")
        wo_sb = const.tile([128, 2 * D], bf16, tag="wo")
        cs_sb = const.tile([128, BS], bf16, tag="cs")
        m4_sb = const.tile([128, 1280], bf16, tag="m4")
        ones_sb = const.tile([128, 128], bf16, tag="ones")
        # Startup loads are queued deep on both HWDGE rings immediately in
        # need order (the SDMA engines pipeline across queued DMAs, so a
        # deep queue drains faster than issue-as-needed). Sync carries the
        # PE-critical wq + xt stream; scalar carries the rest of the
        # weights, then xt tiles 2-3.
        # Early compute order is q(t0), q(t1), k(t0), k(t1), v(t0), v(t1):
        # only wq/xt0/xt1 gate the first ~25us.
        C4 = 4 * 2 * HD  # 4 contraction chunks of a weight matrix, flat
        X4 = 4 * TT      # 4 contraction chunks of an xt tile, flat
        # Three parallel DMA streams (per-ring early throughput is only
        # ~200GB/s, so the 7MB the first 35us of compute needs must be
        # split):
        #   sync (HWDGE):   xt0, xt1 head   -> later all out stores
        #   scalar (HWDGE): wq, cs0, wk, xt1 tail, wv (the weight path)
        #   gpsimd (SWDGE): bulk with slack: m4, cs1, xt2, xt3, wo, xt4-7
        xt_tiles = [
            xtp.tile([128, XCOLS], bf16, tag="xt", name=f"xt{i}") for i in range(4)
        ]
        nc.sync.dma_start(wq_sb[:, 0:C4], wq_h[:, 0:C4])
        nc.sync.dma_start(xt_tiles[0][:, 0:X4], xt_h[:, 0, 0:X4])
        nc.sync.dma_start(wq_sb[:, C4:WCOLS], wq_h[:, C4:WCOLS])
        for p in range(1, 4):
            nc.sync.dma_start(
                xt_tiles[0][:, p * X4 : (p + 1) * X4],
                xt_h[:, 0, p * X4 : (p + 1) * X4],
            )
        for p in range(2):
            nc.sync.dma_start(
                xt_tiles[1][:, p * X4 : (p + 1) * X4],
                xt_h[:, 1, p * X4 : (p + 1) * X4],
            )
        nc.scalar.dma_start(cs_sb[:, 0 : 2 * TT], cs_h[:, 0 : 2 * TT])
        nc.scalar.dma_start(wk_sb[:, 0:C4], wk_h[:, 0:C4])
        nc.scalar.dma_start(wk_sb[:, C4:WCOLS], wk_h[:, C4:WCOLS])
        for p in range(2, 4):
            nc.scalar.dma_start(
                xt_tiles[1][:, p * X4 : (p + 1) * X4],
                xt_h[:, 1, p * X4 : (p + 1) * X4],
            )
        nc.scalar.dma_start(wv_sb[:], wv_h[:])
        nc.scalar.dma_start(m4_sb[:], m4_h[:])
        nc.scalar.dma_start(cs_sb[:, 2 * TT :], cs_h[:, 2 * TT :])
        for t in (2, 3):
            for p in range(2):
                nc.scalar.dma_start(
                    xt_tiles[t][:, p * 2 * X4 : (p + 1) * 2 * X4],
                    xt_h[:, t, p * 2 * X4 : (p + 1) * 2 * X4],
                )
        nc.scalar.dma_start(wo_sb[:], wo_h[:])

        # tiles 4-7 prefetch on scalar, each emitted right after the tile
        # unit that frees its pool slot (so the slot wait is already
        # satisfied and never blocks the ACT stream), keeping the sync ring
        # free for out stores. (SWDGE/gpsimd rings measurably slow the
        # HWDGE rings when used concurrently — keep everything on HWDGE.)
        def prefetch_xt(t):
            xt_t = xtp.tile([128, XCOLS], bf16, tag="xt", name=f"xt{t}")
            assert len(xt_tiles) == t
            xt_tiles.append(xt_t)
            for p in range(2):
                nc.scalar.dma_start(
                    xt_t[:, p * 2 * X4 : (p + 1) * 2 * X4],
                    xt_h[:, t, p * 2 * X4 : (p + 1) * 2 * X4],
                )

        nc.vector.memset(ones_sb[:], 1.0)

        # PE warm-up: the HAM clock gate needs ~3us of sustained matmul
        # activity to lift the PE from 1.2 to 2.4 GHz. The first real
        # matmuls can't start until their DMAs land (~11us), so burn the
        # wait warming the array on the memset ones tile (no DMA deps).
        warm_ps = psS.tile([128, QT], mybir.dt.float32, tag="sc")
        for i in range(16):
            nc.tensor.matmul(
                warm_ps[:, (i % 4) * 128 : (i % 4) * 128 + 128],
                ones_sb[:], ones_sb[:], start=True, stop=True,
            )
        # consume so DCE keeps the warm-up and the PSUM slot is released
        warm_sink = const.tile([1, 8], mybir.dt.float32, tag="wsink")
        nc.scalar.copy(warm_sink[0:1, 0:2], warm_ps[0:1, 0:2])

        # DVE pre-touch of DMA-written constants: TensorTensor instructions
        # encode only one sync-wait slot, so the DVE vector clock must have
        # observed these DMAs before any TT reads them (else walrus dies with
        # "Too many sync wait commands").
        scratch = const.tile([1, 8], bf16, tag="scratch")
        nc.vector.tensor_copy(scratch[0:1, 0:2], cs_sb[0:1, 0:2])
        nc.vector.tensor_copy(scratch[0:1, 2:4], m4_sb[0:1, 0:2])

        # persistent activations
        qr = persist.tile([128, HPC, BS], bf16, tag="qr")   # rotated qT per head
        kr = persist.tile([128, HPC, BS], bf16, tag="kr")   # rotated kT per head
        v_sb = persist.tile([128, BS // 128, 2 * HD], bf16, tag="v")  # tok-major v
        on_sb = persist.tile([128, HPC, B, S], bf16, tag="on")  # normalized oT

        # ---- phase 1 unit: one token tile's q/k/v projections + RoPE ----
        alt = [0]

        def tile_unit(t, parts="qkv"):
            t0 = t * TT
            xt_t = xt_tiles[t]

            # During tiles 0-3 the score pool psS is idle: alternate the
            # projection PSUM between psA and psS so a slow RoPE drain
            # (DVE, gated on the cs DMA) can lag 4 groups behind the PE
            # without starving it of accumulator slots.
            def proj_tile():
                alt[0] += 1
                if alt[0] % 2:
                    return psS.tile([128, TT], mybir.dt.float32, tag="sc", name="pp")
                return psA.tile([128, TT], mybir.dt.float32, tag="proj", name="pp")

            co = cs_sb[0:HALF, t0 : t0 + TT]
            si = cs_sb[HALF:128, t0 : t0 + TT]
            # q for both heads first (only needs wq), then k for both
            # heads: gives the wk DMA (behind wq on sync) time to land
            # without stalling the PE on tile 0.
            wlist = []
            if "q" in parts:
                wlist.append((wq_sb, qr))
            if "k" in parts:
                wlist.append((wk_sb, kr))
            for w_sb, dstT in wlist:
                for h in range(HPC):
                    pq = proj_tile()
                    for c in range(NKT):
                        wc = c * 2 * HD + h * HD
                        nc.tensor.matmul(
                            pq[:],
                            w_sb[:, wc : wc + HD],
                            xt_t[:, c * TT : (c + 1) * TT],
                            start=(c == 0),
                            stop=(c == NKT - 1),
                        )
                    t1 = ropet.tile([HALF, TT], bf16, tag="rt")
                    t2 = ropet.tile([HALF, TT], bf16, tag="rt")
                    t3 = ropet.tile([HALF, TT], bf16, tag="rt")
                    t4 = ropet.tile([HALF, TT], bf16, tag="rt")
                    # all four PSUM-reading muls first so pq's slot frees
                    # as soon as possible (the combines only read SBUF)
                    nc.vector.tensor_tensor(t1[:], pq[0:HALF, :], co, mult)
                    nc.vector.tensor_tensor(t2[:], pq[HALF:128, :], si, mult)
                    nc.vector.tensor_tensor(t3[:], pq[0:HALF, :], si, mult)
                    nc.vector.tensor_tensor(t4[:], pq[HALF:128, :], co, mult)
                    # combines are SBUF-only and far off the critical path
                    # (consumed by attention much later): run them on the
                    # otherwise-idle Pool engine to unload DVE
                    nc.gpsimd.tensor_tensor(
                        dstT[0:HALF, h, t0 : t0 + TT], t1[:], t2[:], sub
                    )
                    nc.gpsimd.tensor_tensor(
                        dstT[HALF:128, h, t0 : t0 + TT], t3[:], t4[:], add
                    )

            # v projection, token-major [tok, 2*HD]
            for m in range(TT // 128 if "v" in parts else 0):
                pv = proj_tile()
                for c in range(NKT):
                    nc.tensor.matmul(
                        pv[:, 0 : 2 * HD],
                        xt_t[:, c * TT + m * 128 : c * TT + (m + 1) * 128],
                        wv_sb[:, c * 2 * HD : (c + 1) * 2 * HD],
                        start=(c == 0),
                        stop=(c == NKT - 1),
                    )
                g = t * (TT // 128) + m
                nc.scalar.copy(v_sb[:, g, :], pv[:, 0 : 2 * HD])

        # out-proj work queue: single [128,512] units threaded into the
        # attention chunk loop so the PE stream never has copy-gated bursts
        pend = []

        def emit_pend(n):
            for _ in range(min(n, len(pend))):
                outproj_one(*pend.pop(0))

        def attn(b, h, qt, reserve=0):
            q0 = b * S + qt * QT
            # k chunks packed two per PSUM tile / exp call (halves the ACT
            # instruction count and its per-call PSUM access penalty).
            # Full-width unmasked pairs below the diagonal first, then the
            # two triangularly-trimmed masked diagonal pairs, whose chunks
            # pack at column offsets c0 matching the m4 mask layout.
            pairs = [
                ([(2 * p, 0, QT, 0), (2 * p + 1, 0, QT, QT)], None)
                for p in range(2 * qt)
            ]
            pairs.append(
                ([(4 * qt, 0, QT, 0), (4 * qt + 1, 128, QT - 128, QT)], 0)
            )
            pairs.append(
                ([(4 * qt + 2, 256, QT - 256, 0),
                  (4 * qt + 3, 384, QT - 384, QT - 256)], 896)
            )
            n = 4 * qt + 4
            ov = psO.tile([128, QT], mybir.dt.float32, tag="ov")
            es = esp.tile([128, QT], bf16, tag="es")
            meta = []
            flat = [0]

            def emit_pair(pcs, moff):
                wtot = pcs[-1][3] + pcs[-1][2]
                sc = psS.tile([128, 2 * QT], mybir.dt.float32, tag="sc", name="sc")
                for kb, qoff, w, c0 in pcs:
                    k0 = b * S + kb * KC
                    nc.tensor.matmul(
                        sc[:, c0 : c0 + w],
                        kr[:, h, k0 : k0 + KC],
                        qr[:, h, q0 + qoff : q0 + QT],
                        start=True,
                        stop=True,
                    )
                e = expp.tile([128, 2 * QT], bf16, tag="e", name="e")
                nc.scalar.activation(e[:, 0:wtot], sc[:, 0:wtot], Exp, scale=ISQRT)
                if moff is not None:
                    nc.vector.tensor_tensor(
                        e[:, 0:wtot], e[:, 0:wtot], m4_sb[:, moff : moff + wtot], mult
                    )
                ms = []
                for kb, qoff, w, c0 in pcs:
                    if flat[0] == 0:
                        nc.vector.tensor_copy(es[:], e[:, 0:QT])
                    else:
                        nc.vector.tensor_tensor(
                            es[:, qoff:QT], es[:, qoff:QT], e[:, c0 : c0 + w], add
                        )
                    ms.append((kb, qoff, w, c0, flat[0]))
                    flat[0] += 1
                meta.append((e, ms))

            def do_pv(pi):
                e, ms = meta[pi]
                for kb, qoff, w, c0, ci in ms:
                    gk = (b * S + kb * KC) // 128
                    nc.tensor.matmul(
                        ov[:, qoff:QT],
                        v_sb[:, gk, h * HD : (h + 1) * HD],
                        e[:, c0 : c0 + w],
                        start=(ci == 0),
                        stop=(ci == n - 1),
                        skip_group_check=True,
                    )

            # PV lags TWO pairs behind the score/exp front (expp holds 4
            # e tiles), so a PV matmul's exp+mask have ~2 pair-times of
            # slack and the PE stops catching the ACT chain's tail
            for pi, (pcs, moff) in enumerate(pairs):
                emit_pair(pcs, moff)
                if pi >= 2:
                    do_pv(pi - 2)
                if len(pend) > reserve:
                    emit_pend(min(2, len(pend) - reserve))
            if len(pairs) >= 2:
                do_pv(len(pairs) - 2)
            do_pv(len(pairs) - 1)
            # sum over k (partition axis) via ones-matmul: reduces and
            # broadcasts the denominator to all 128 partitions in one shot
            sm = psA.tile([128, QT], mybir.dt.float32, tag="proj", name="sm")
            nc.tensor.matmul(sm[:], ones_sb[:], es[:], start=True, stop=True)
            # fill the serial ones-mm -> Ln -> Exp -> mult latency with any
            # queued out-proj units (they depend only on earlier attns)
            emit_pend(4)
            # 1/sum via exp(-ln(sum)): Ln and Exp share one ACT table so no
            # table reloads (DVE has no divide, and TT can't read 2x PSUM).
            lnt = esp.tile([128, QT], mybir.dt.float32, tag="smsb")
            nc.scalar.activation(lnt[:], sm[:], Ln)
            rr = esp.tile([128, QT], mybir.dt.float32, tag="smsb")
            nc.scalar.activation(rr[:], lnt[:], Exp, scale=-1.0)
            # pre-touch rr on DVE so the norm TT only waits on PE
            nc.vector.tensor_copy(scratch[0:1, 4:6], rr[0:1, 0:2])
            nc.vector.tensor_tensor(
                on_sb[:, h, b, qt * QT : qt * QT + QT], ov[:], rr[:], mult
            )

        ecount = 0
        cur_stage = [None]
        # During the tiles 4-7 stretch ACT has no exp work but DVE carries
        # all the RoPE muls; copies landing on DVE there queue ahead of the
        # muls that must drain the projection PSUM, stalling PE group
        # starts. Force copies to ACT in that phase.
        force_copy = [None]

        def outproj_one(b, tcn, et):
            nonlocal ecount
            po = psA.tile([128, 512], mybir.dt.float32, tag="proj")
            for j in range(HPC):
                nc.tensor.matmul(
                    po[:],
                    on_sb[:, j, b, tcn * 128 : tcn * 128 + 128],
                    wo_sb[:, j * D + et * 512 : j * D + et * 512 + 512],
                    start=(j == 0),
                    stop=(j == HPC - 1),
                )
            if et == 0:
                cur_stage[0] = outp.tile([128, D], f16, tag="ob", name="ob")
            ob = cur_stage[0]
            if force_copy[0] is not None:
                eng = force_copy[0]
            else:
                eng = (nc.scalar.copy, nc.vector.tensor_copy)[ecount % 2]
            eng(ob[:, et * 512 : et * 512 + 512], po[:])
            ecount += 1
            rows = slice(b * S + tcn * 128, b * S + tcn * 128 + 128)
            if b == 1 and tcn >= 12:
                # tail-critical blocks: store per half; the final block
                # per-unit so the very last transfer is only 128KB
                if tcn == 15:
                    cols = slice(et * 512, et * 512 + 512)
                    nc.sync.dma_start(out_h[rows, cols], ob[:, cols])
                elif et == 1 or et == 3:
                    cols = slice(0, 1024) if et == 1 else slice(1024, 2048)
                    nc.sync.dma_start(out_h[rows, cols], ob[:, cols])
            elif et == 3:
                # one full-width store per token block: 4KB contiguous rows
                # drain the ring at roughly twice the rate of 2KB halves
                nc.sync.dma_start(out_h[rows, :], ob[:, :])

        # Batch-0 token tiles first; then batch-1 tiles with batch-0's
        # attention interleaved between them (its exp load lands where
        # ScalarE is otherwise idle, and it needs no PSUM "proj" slots so
        # it doesn't fight the projection groups). Out-proj units run as
        # single [128,512] groups threaded into the batch-1 attention
        # chunk loops (emit_pend inside attn), never in bursts.
        # Batch-0 attention q-tile t only needs tokens [0, (t+1)*512) of
        # q/k/v — run it right after token tile t, where its matmuls and
        # exps fill the startup DMA-delivery stalls. Tiles 4-7 (batch 1
        # projections) then run as one PE-saturated stretch, draining
        # queued out-proj units into any gaps.
        # tiles 0-1 run q(t0), k(t0), q(t1), k(t1): tile 0's work only
        # needs wq/wk/xt0 (first on both rings), so the PE never idles on
        # xt1's tail the way q(t0),q(t1) back-to-back would
        tile_unit(0, parts="q")
        tile_unit(0, parts="k")
        tile_unit(1, parts="q")
        tile_unit(1, parts="k")
        tile_unit(0, parts="v")
        prefetch_xt(4)
        tile_unit(1, parts="v")
        prefetch_xt(5)
        for t in range(2):
            attn(0, 0, t, reserve=6)
            attn(0, 1, t, reserve=6)
            pend += [(0, 4 * t + i, et) for i in range(4) for et in range(D // 512)]
        for t in range(2, NTT // 2):
            tile_unit(t)
            prefetch_xt(t + 4)
            attn(0, 0, t, reserve=6)
            attn(0, 1, t, reserve=6)
            pend += [(0, 4 * t + i, et) for i in range(4) for et in range(D // 512)]
        force_copy[0] = nc.scalar.copy
        for t in range(NTT // 2, NTT):
            tile_unit(t)
            emit_pend(4)
        force_copy[0] = None
        # a standing reserve of units keeps the PE fed through every
        # ones-mm -> Ln -> Exp -> mult boundary chain in the b1 phase
        for qt in range(S // QT):
            attn(1, 0, qt, reserve=12)
            emit_pend(2)
            attn(1, 1, qt, reserve=12)
            emit_pend(2)
            pend += [(1, 4 * qt + i, et) for i in range(4) for et in range(D // 512)]
        emit_pend(len(pend))
    return _patch_serialization(nc)


def _prep_inputs(x, wq, wk, wv, wo, freqs_cos, freqs_sin):
    import ml_dtypes

    bf16 = ml_dtypes.bfloat16
    perm = np.concatenate([np.arange(0, HD, 2), np.arange(1, HD, 2)])

    xt = np.ascontiguousarray(x.reshape(BS, D).T)          # [D, BS]
    # tile-major pack: [128, NTT, NKT*TT] so a multi-chunk DMA piece of one
    # token tile is contiguous per partition (4-16KB DMA elements)
    xt_r = np.ascontiguousarray(
        xt.reshape(NKT, 128, NTT, TT).transpose(1, 2, 0, 3).reshape(
            128, NTT, NKT * TT
        )
    ).astype(bf16)

    cosT = freqs_cos.T.astype(np.float32)                  # [64, S]
    sinT = freqs_sin.T.astype(np.float32)
    cs = np.concatenate(
        [np.tile(cosT, (1, B)), np.tile(sinT, (1, B))], axis=0
    ).astype(bf16)                                         # [128, BS]

    i = np.arange(KC)[:, None]
    m4 = np.concatenate(
        [(i <= np.arange(w)[None, :]).astype(np.float32) for w in (512, 384, 256, 128)],
        axis=1,
    ).astype(bf16)                                         # [128, 1280] packed triangles

    def pack_w(wmat_cols):
        # wmat_cols: [D, 2*HD] -> [128, NKT * 2*HD] (flat chunk-major)
        return np.ascontiguousarray(
            wmat_cols.reshape(NKT, 128, 2 * HD).transpose(1, 0, 2).reshape(
                128, NKT * 2 * HD
            )
        ).astype(bf16)

    in_maps = []
    for c in range(NCORES):
        heads = [HPC * c + hh for hh in range(HPC)]
        wq_c = np.concatenate(
            [wq[h * HD : (h + 1) * HD][perm].T for h in heads], axis=1
        )                                                  # [D, 2*HD]
        wk_c = np.concatenate(
            [wk[h * HD : (h + 1) * HD][perm].T for h in heads], axis=1
        )
        wv_c = np.concatenate(
            [wv[h * HD : (h + 1) * HD].T for h in heads], axis=1
        )
        wo_c = np.stack(
            [wo[:, h * HD : (h + 1) * HD].T for h in heads], axis=0
        )                                                  # [2, HD, D]
        wo_r = np.ascontiguousarray(wo_c.transpose(1, 0, 2)).astype(bf16).reshape(
            128, 2 * D
        )
        in_maps.append(
            dict(
                xt=xt_r,
                wq=pack_w(wq_c),
                wk=pack_w(wk_c),
                wv=pack_w(wv_c),
                wo=wo_r,
                cs=cs,
                m4=m4,
            )
        )
    return in_maps


_NC_CACHE = {}


def kernel(x, wq, wk, wv, wo, freqs_cos, freqs_sin, mask):
    from concourse.bass_utils import run_bass_kernel_spmd

    in_maps = _prep_inputs(x, wq, wk, wv, wo, freqs_cos, freqs_sin)
    if "nc" not in _NC_CACHE:
        _NC_CACHE["nc"] = _build_nc()
    nc = _NC_CACHE["nc"]
    res = run_bass_kernel_spmd(nc, in_maps, core_ids=list(range(NCORES)))
    parts = [r["out"].astype(np.float32) for r in res.results]
    out = np.sum(np.stack(parts, 0), axis=0, dtype=np.float32)
    return out.reshape(B, S, D)

